# revision 18
# baseline (speedup 1.0000x reference)
"""Trainium2 Bass kernel for 16-head MHA (B=2, S=2048, D=1024, E=64).

Sharding: 8 cores = 2 batches x 4 head-groups. Each core computes 4 heads
(2 pairs of 2) for one batch and returns a partial output [2048, 1024]
(sum of its 4 heads' contributions after the output projection). Host sums
the 4 partials per batch.

Per-core pipeline (all matmuls on PE, fp32 PSUM accumulation):
  - K/Q projections feature-major (weights stationary, x moving)
  - V projection token-major directly on the PE (x chunk stationary,
    W_val moving) -- avoids DMA transposes entirely
  - S^T = K Q^T per head pair, two heads row-packed in the 128x128 array
  - A^T = exp(S^T) on ACT (scale folded into W_query on host); ACT does
    ONLY exp -- all psum evacuations go through DVE
  - O^T accumulation with fused row-sum via a ones column in the V tiles
  - softmax normalization: DVE reciprocal_approx_fast + GPSIMD
    partition-broadcast + DVE multiply (writes fp16 O^T)
  - output projection (fp16) accumulating both pairs, fp16 partials out
  - phase 1 is software-pipelined into attention: slice-ordered DMAs,
    K proj first, V-blocks + Q-slices interleaved with attention qc=0
"""

import sys

sys.path.insert(0, "/opt/trn_rl_repo")

import numpy as np

import concourse.bass as bass
import concourse.bacc as bacc
import concourse.mybir as mybir
from concourse import tile
from concourse.tile_rust import add_dep_helper
from concourse.bass_interp import get_hw_module
from concourse.bass_utils import run_bass_kernel_spmd

F16 = mybir.dt.float16
F32 = mybir.dt.float32
BF16 = mybir.dt.bfloat16
I16 = mybir.dt.int16

# Schraudolph exp: bf16 bits = round(x * 128/ln2 + B); B tuned for zero mean
# relative error so softmax numerator/denominator biases cancel
SCHRAUD_A = float(np.float32(128.0 / np.log(2.0)))
SCHRAUD_B = 16250.0

N_CORES = 8
T = 2048          # tokens per core (one batch)
D = 1024          # model dim
E = 64            # head dim
QC = 512          # query chunk
NQ = T // QC      # 4 query chunks
KB = 128          # key block
NKB = T // KB     # 16 key blocks
ND = D // 128     # 8 contraction chunks for projections
NS = 4            # token slices (512 each)

_CACHE = {}


def _build():
    nc = bacc.Bacc("TRN2", target_bir_lowering=False, debug=False,
                   num_devices=N_CORES)

    xqT = nc.dram_tensor("xqT", [D, T], F16, kind="ExternalInput").ap()
    xkT = nc.dram_tensor("xkT", [D, T], F16, kind="ExternalInput").ap()
    xvT = nc.dram_tensor("xvT", [D, T], F16, kind="ExternalInput").ap()
    # per-pair packed weights, layout [128, 8*128]: chunk d at cols d*128
    wq = [nc.dram_tensor(f"wq{p}", [128, D], F16, kind="ExternalInput").ap()
          for p in range(2)]
    wk = [nc.dram_tensor(f"wk{p}", [128, D], F16, kind="ExternalInput").ap()
          for p in range(2)]
    # V weights as moving operand: [128, 8*256]; chunk d at cols d*256,
    # within a chunk cols h*64:(h+1)*64 = head h features
    wvm = nc.dram_tensor("wvm", [128, 8 * 256], F16, kind="ExternalInput").ap()
    wo = [nc.dram_tensor(f"wo{p}", [128, D], F16, kind="ExternalInput").ap()
          for p in range(2)]
    pout = nc.dram_tensor("pout", [T, D], F16, kind="ExternalOutput").ap()

    with tile.TileContext(nc) as tc:
        with (
            tc.tile_pool(name="consts", bufs=1) as consts,
            tc.tile_pool(name="persist", bufs=1) as persist,
            tc.tile_pool(name="xs", bufs=1) as xs,
            tc.tile_pool(name="at", bufs=5) as atp,
            tc.tile_pool(name="o2t", bufs=2) as o2tp,
            tc.tile_pool(name="os", bufs=3) as osp,
            tc.tile_pool(name="small", bufs=2) as smallp,
            tc.tile_pool(name="psS", bufs=2, space="PSUM") as psS,
            tc.tile_pool(name="psO", bufs=1, space="PSUM") as psO,
            tc.tile_pool(name="psP", bufs=2, space="PSUM") as psP,
        ):
            # ---- weights ----
            wq_sb = [consts.tile([128, D], F16, tag=f"wq{p}", name=f"wq_sb{p}") for p in range(2)]
            wk_sb = [consts.tile([128, D], F16, tag=f"wk{p}", name=f"wk_sb{p}") for p in range(2)]
            wv_sb = consts.tile([128, 8 * 256], F16, tag="wvm", name="wv_sb")
            wo_sb = [consts.tile([128, D], F16, tag=f"wo{p}", name=f"wo_sb{p}") for p in range(2)]
            for p in range(2):
                nc.sync.dma_start(wq_sb[p][:], wq[p][:])
                nc.sync.dma_start(wk_sb[p][:], wk[p][:])
                nc.sync.dma_start(wo_sb[p][:], wo[p][:])
            nc.sync.dma_start(wv_sb[:], wvm[:])

            # ---- persistent activations ----
            # feature-major Q^T, K^T per pair: rows 0:64 head-even, 64:128 head-odd
            qt = [[persist.tile([128, QC], F16, tag=f"qt{p}_{t}", name=f"qt{p}_{t}")
                   for t in range(NQ)] for p in range(2)]
            kt = [persist.tile([128, T], F16, tag=f"kt{p}", name=f"kt{p}") for p in range(2)]
            # token-major [V_even | 1 | V_odd | 1] per (pair, key-block): [128, 130]
            v2 = [[persist.tile([128, 130], BF16, tag=f"v2_{p}_{b}", name=f"v2_{p}_{b}")
                   for b in range(NKB)] for p in range(2)]
            for p in range(2):
                for b in range(NKB):
                    nc.vector.memset(
                        v2[p][b].rearrange("p (c n) -> p c n", c=2)[:, :, 64:65], 1.0)

            # ---- input DMAs: full [128, T] chunks, ordered k, q, v ----
            def load_chunks(x_dram, pfx):
                ts = []
                for d in range(ND):
                    t = xs.tile([128, T], F16, tag=f"{pfx}{d}", name=f"{pfx}{d}")
                    nc.sync.dma_start(t[:], x_dram[d * 128:(d + 1) * 128, :])
                    ts.append(t)
                return ts

            xk_t = load_chunks(xkT, "xk")
            xq_t = load_chunks(xqT, "xq")
            xv_t = load_chunks(xvT, "xv")

            # ---- projection helpers ----
            def proj_fm_slice(x_tiles, w_sb, s, evac):
                # feature-major: weights stationary, x moving; psum per pair
                for p in range(2):
                    ps = psP.tile([128, QC], F32, tag="pp", name=f"pj_{id(x_tiles)}_{s}_{p}")
                    for d in range(ND):
                        nc.tensor.matmul(
                            ps[:], w_sb[p][:, d * 128:(d + 1) * 128],
                            x_tiles[d][:, s * QC:(s + 1) * QC],
                            start=(d == 0), stop=(d == ND - 1))
                    evac(p, s, ps)

            def evac_kt(p, s, ps):
                nc.vector.tensor_copy(kt[p][:, s * QC:(s + 1) * QC], ps[:])

            def evac_qt(p, s, ps):
                nc.vector.tensor_copy(qt[p][s][:], ps[:])

            def proj_v_block(b):
                # token-major V: x chunk slice stationary, W_val moving
                ps = psP.tile([128, 256], F32, tag="pp", name=f"pv_{b}")
                for d in range(ND):
                    nc.tensor.matmul(
                        ps[:],
                        xv_t[d][:, b * 128:(b + 1) * 128],
                        wv_sb[:, d * 256:(d + 1) * 256],
                        start=(d == 0), stop=(d == ND - 1))
                for p in range(2):
                    nc.vector.tensor_copy(
                        v2[p][b].rearrange("p (c n) -> p c n", c=2)[:, :, 0:64],
                        ps[:, p * 128:(p + 1) * 128].rearrange(
                            "p (c n) -> p c n", c=2))

            # ---- attention + output projection ----
            ost_live = {}

            def emit_outproj_group(qc, o2t, sub, oc, anchor):
                q0 = qc * QC
                if oc == 0:
                    ost_live[(qc, sub)] = osp.tile(
                        [128, D], F16, tag="os", name=f"os_{qc}_{sub}")
                ost = ost_live[(qc, sub)]
                pp = psP.tile([128, 512], F32, tag="pp", name=f"pp_{qc}_{sub}_{oc}")
                for p in range(2):
                    mm = nc.tensor.matmul(
                        pp[:],
                        o2t[p][:, sub * 128:(sub + 1) * 128],
                        wo_sb[p][:, oc * 512:(oc + 1) * 512],
                        start=(p == 0), stop=(p == 1))
                    if p == 0 and anchor is not None:
                        add_dep_helper(mm.ins, anchor.ins, sync=False,
                                       reason="interleave outproj after S")
                nc.scalar.activation(
                    ost[:, oc * 512:(oc + 1) * 512], pp[:],
                    mybir.ActivationFunctionType.Copy)
                if oc == 1:
                    nc.sync.dma_start(
                        pout[q0 + sub * 128:q0 + (sub + 1) * 128, :],
                        ost[:])
                    del ost_live[(qc, sub)]

            def emit_outproj(qc, o2t, anchor=None):
                for sub in range(4):
                    for oc in range(2):
                        emit_outproj_group(qc, o2t, sub, oc, anchor)

            po_live = {}

            def attn_kb(qc, p, kb, pending):
                # S^T then exp then O^T accumulation for one key block
                k0 = kb * KB
                ps = psS.tile([128, 2 * QC], F32, tag="s", name=f"s_{qc}_{p}_{kb}")
                s_anchor = nc.tensor.matmul(
                    ps[:, 0:QC],
                    kt[p][0:64, k0:k0 + KB],
                    qt[p][qc][0:64, :],
                    start=True, stop=True, tile_position=(0, 0))
                nc.tensor.matmul(
                    ps[:, QC:2 * QC],
                    kt[p][64:128, k0:k0 + KB],
                    qt[p][qc][64:128, :],
                    start=True, stop=True, tile_position=(64, 0))
                at = atp.tile([128, 2 * QC], BF16, tag="at", name=f"at_{qc}_{p}_{kb}")
                if kb % 2 == 1:
                    # DVE one-op Schraudolph exp (round-to-nearest int16 cast,
                    # bitcast to bf16); splits exp work off the ACT engine
                    nc.vector.tensor_scalar(
                        out=at.bitcast(I16)[:], in0=ps[:],
                        scalar1=SCHRAUD_A, scalar2=SCHRAUD_B,
                        op0=mybir.AluOpType.mult, op1=mybir.AluOpType.add)
                else:
                    nc.scalar.activation(
                        at[:], ps[:], mybir.ActivationFunctionType.Exp)
                po = po_live[(qc, p)]
                for h in range(2):
                    nc.tensor.matmul(
                        po[h][:],
                        v2[p][kb][:, h * 65:h * 65 + 65],
                        at[:, h * QC:(h + 1) * QC],
                        start=(kb == 0), stop=(kb == NKB - 1))
                if p == 1 and pending is not None and kb % 2 == 1:
                    pqc, po2t = pending
                    emit_outproj_group(pqc, po2t, kb // 4, (kb // 2) % 2,
                                       s_anchor)

            def attn_norm(qc, p, o2t):
                # softmax normalization for both heads of pair p
                # (po rows 0:64 = O^T, row 64 = denominator)
                po = po_live.pop((qc, p))
                for h in range(2):
                    ot = smallp.tile([64, QC], F32, tag=f"ot{h}", name=f"ot_{qc}_{p}_{h}")
                    nc.vector.tensor_copy(ot[:], po[h][0:64, :])
                    den = smallp.tile([1, QC], F32, tag=f"den{h}", name=f"den_{qc}_{p}_{h}")
                    nc.vector.tensor_copy(den[:], po[h][64:65, :])
                    r = smallp.tile([1, QC], F32, tag=f"r{h}", name=f"r_{qc}_{p}_{h}")
                    nc.vector.reciprocal_approx_fast(r[:], den[:])
                    rb = smallp.tile([64, QC], F32, tag=f"rb{h}", name=f"rb_{qc}_{p}_{h}")
                    nc.gpsimd.partition_broadcast(rb[:], r[:])
                    nc.gpsimd.tensor_mul(
                        o2t[h * 64:(h + 1) * 64, :],
                        ot[:], rb[:])

            def new_po(qc, p):
                po_live[(qc, p)] = [
                    psO.tile([65, QC], F32, tag=f"o{h}", name=f"po_{qc}_{p}_{h}")
                    for h in range(2)]

            # ---- emission schedule ----
            # K projection first (all slices)
            for s in range(NS):
                proj_fm_slice(xk_t, wk_sb, s, evac_kt)

            # qc=0 attention (pair 0) interleaved with V blocks and Q slices
            o2t_cur = [o2tp.tile([128, QC], F16, tag=f"o2t{p}", name=f"o2t_0_{p}")
                       for p in range(2)]
            new_po(0, 0)
            for g in range(4):
                if g == 0:
                    proj_fm_slice(xq_t, wq_sb, 0, evac_qt)
                for b in range(4 * g, 4 * g + 4):
                    proj_v_block(b)
                if g > 0:
                    proj_fm_slice(xq_t, wq_sb, g, evac_qt)
                for kb in range(4 * g, 4 * g + 4):
                    attn_kb(0, 0, kb, None)
            attn_norm(0, 0, o2t_cur[0])
            new_po(0, 1)
            for kb in range(NKB):
                attn_kb(0, 1, kb, None)
            attn_norm(0, 1, o2t_cur[1])
            pending = (0, o2t_cur)

            # qc=1..3 with previous qc's outproj interleaved into pair 1
            for qc in range(1, NQ):
                o2t_cur = [o2tp.tile([128, QC], F16, tag=f"o2t{p}", name=f"o2t_{qc}_{p}")
                           for p in range(2)]
                for p in range(2):
                    new_po(qc, p)
                    for kb in range(NKB):
                        attn_kb(qc, p, kb, pending if p == 1 else None)
                    attn_norm(qc, p, o2t_cur[p])
                pending = (qc, o2t_cur)
            emit_outproj(*pending)

    nc.compile()
    nc.m = get_hw_module(nc.m)
    return nc


def _pack_w(w_pair):
    # w_pair: [2, 1024, 64] -> [1024, 128] -> chunk-major [128, 8*128]
    w = np.concatenate([w_pair[0], w_pair[1]], axis=1)          # [1024, 128]
    return np.ascontiguousarray(
        w.reshape(ND, 128, 128).transpose(1, 0, 2).reshape(128, D))


def _pack_wv(w4):
    # w4: [4, 1024, 64] -> [1024, 256] -> chunk-major [128, 8*256]
    w = np.concatenate([w4[h] for h in range(4)], axis=1)       # [1024, 256]
    return np.ascontiguousarray(
        w.reshape(ND, 128, 256).transpose(1, 0, 2).reshape(128, ND * 256))


def _pack_wo(wo_pair):
    # wo_pair: [2, 64, 1024] -> [128, 1024]
    return np.ascontiguousarray(np.concatenate([wo_pair[0], wo_pair[1]], axis=0))


def kernel(q, k, v, W_query, W_key, W_val, W_out, _trace=False):
    q = np.asarray(q, dtype=np.float32)
    k = np.asarray(k, dtype=np.float32)
    v = np.asarray(v, dtype=np.float32)
    W_query = np.asarray(W_query, dtype=np.float32)
    W_key = np.asarray(W_key, dtype=np.float32)
    W_val = np.asarray(W_val, dtype=np.float32)
    W_out = np.asarray(W_out, dtype=np.float32)

    if "nc" not in _CACHE:
        _CACHE["nc"] = _build()
    nc = _CACHE["nc"]

    norm = 1.0 / np.sqrt(E)
    xT = {}
    for b in range(2):
        xT[("q", b)] = np.ascontiguousarray(q[b].T).astype(np.float16)
        xT[("k", b)] = np.ascontiguousarray(k[b].T).astype(np.float16)
        xT[("v", b)] = np.ascontiguousarray(v[b].T).astype(np.float16)

    in_maps = []
    for c in range(N_CORES):
        b, g = c // 4, c % 4
        hs = [4 * g, 4 * g + 1, 4 * g + 2, 4 * g + 3]
        m = {
            "xqT": xT[("q", b)], "xkT": xT[("k", b)], "xvT": xT[("v", b)],
            "wvm": _pack_wv(W_val[hs]).astype(np.float16),
        }
        for p in range(2):
            hp = hs[2 * p:2 * p + 2]
            m[f"wq{p}"] = _pack_w(W_query[hp] * norm).astype(np.float16)
            m[f"wk{p}"] = _pack_w(W_key[hp]).astype(np.float16)
            m[f"wo{p}"] = _pack_wo(W_out[hp]).astype(np.float16)
        in_maps.append(m)

    res = run_bass_kernel_spmd(nc, in_maps, list(range(N_CORES)),
                               trace=_trace)
    parts = [res.results[c]["pout"].astype(np.float32) for c in range(N_CORES)]
    out = np.stack([
        parts[0] + parts[1] + parts[2] + parts[3],
        parts[4] + parts[5] + parts[6] + parts[7],
    ]).astype(np.float32)
    if _trace:
        _CACHE["last_result"] = res
    return out


# revision 21
# speedup vs baseline: 1.0266x; 1.0266x over previous
"""Trainium2 Bass kernel for 16-head MHA (B=2, S=2048, D=1024, E=64).

Sharding: 8 cores = 2 batches x 4 head-groups. Each core computes 4 heads
(2 pairs of 2) for one batch and returns a partial output [2048, 1024]
(sum of its 4 heads' contributions after the output projection). Host sums
the 4 partials per batch.

Per-core pipeline (all matmuls on PE, fp32 PSUM accumulation):
  - K/Q projections feature-major (weights stationary, x moving)
  - V projection token-major directly on the PE (x chunk stationary,
    W_val moving) -- avoids DMA transposes entirely
  - S^T = K Q^T per head pair, two heads row-packed in the 128x128 array
  - A^T = exp(S^T) on ACT (scale folded into W_query on host); ACT does
    ONLY exp -- all psum evacuations go through DVE
  - O^T accumulation with fused row-sum via a ones column in the V tiles
  - softmax normalization: DVE reciprocal_approx_fast + GPSIMD
    partition-broadcast + DVE multiply (writes fp16 O^T)
  - output projection (fp16) accumulating both pairs, fp16 partials out
  - phase 1 is software-pipelined into attention: slice-ordered DMAs,
    K proj first, V-blocks + Q-slices interleaved with attention qc=0
"""

import sys

sys.path.insert(0, "/opt/trn_rl_repo")

import numpy as np

import concourse.bass as bass
import concourse.bacc as bacc
import concourse.mybir as mybir
from concourse import tile
from concourse.tile_rust import add_dep_helper
from concourse.bass_interp import get_hw_module
from concourse.bass_utils import run_bass_kernel_spmd

F16 = mybir.dt.float16
F32 = mybir.dt.float32
BF16 = mybir.dt.bfloat16
I16 = mybir.dt.int16

# Schraudolph exp: bf16 bits = round(x * 128/ln2 + B); B tuned for zero mean
# relative error so softmax numerator/denominator biases cancel
SCHRAUD_A = float(np.float32(128.0 / np.log(2.0)))
SCHRAUD_B = 16250.0

N_CORES = 8
T = 2048          # tokens per core (one batch)
D = 1024          # model dim
E = 64            # head dim
QC = 512          # query chunk
NQ = T // QC      # 4 query chunks
KB = 128          # key block
NKB = T // KB     # 16 key blocks
ND = D // 128     # 8 contraction chunks for projections
NS = 4            # token slices (512 each)

_CACHE = {}


def _build():
    nc = bacc.Bacc("TRN2", target_bir_lowering=False, debug=False,
                   num_devices=N_CORES)

    xqT = nc.dram_tensor("xqT", [D, T], F16, kind="ExternalInput").ap()
    xkT = nc.dram_tensor("xkT", [D, T], F16, kind="ExternalInput").ap()
    xvT = nc.dram_tensor("xvT", [D, T], F16, kind="ExternalInput").ap()
    # per-pair packed weights, layout [128, 8*128]: chunk d at cols d*128
    wq = [nc.dram_tensor(f"wq{p}", [128, D], F16, kind="ExternalInput").ap()
          for p in range(2)]
    wk = [nc.dram_tensor(f"wk{p}", [128, D], F16, kind="ExternalInput").ap()
          for p in range(2)]
    # V weights as moving operand: [128, 8*256]; chunk d at cols d*256,
    # within a chunk cols h*64:(h+1)*64 = head h features
    wvm = nc.dram_tensor("wvm", [128, 8 * 256], F16, kind="ExternalInput").ap()
    wo = [nc.dram_tensor(f"wo{p}", [128, D], F16, kind="ExternalInput").ap()
          for p in range(2)]
    pout = nc.dram_tensor("pout", [T, D], F16, kind="ExternalOutput").ap()

    with tile.TileContext(nc) as tc:
        with (
            tc.tile_pool(name="consts", bufs=1) as consts,
            tc.tile_pool(name="persist", bufs=1) as persist,
            tc.tile_pool(name="xs", bufs=1) as xs,
            tc.tile_pool(name="at", bufs=5) as atp,
            tc.tile_pool(name="o2t", bufs=2) as o2tp,
            tc.tile_pool(name="os", bufs=3) as osp,
            tc.tile_pool(name="small", bufs=2) as smallp,
            tc.tile_pool(name="psS", bufs=2, space="PSUM") as psS,
            tc.tile_pool(name="psO", bufs=1, space="PSUM") as psO,
            tc.tile_pool(name="psP", bufs=2, space="PSUM") as psP,
        ):
            # ---- weights ----
            wq_sb = [consts.tile([128, D], F16, tag=f"wq{p}", name=f"wq_sb{p}") for p in range(2)]
            wk_sb = [consts.tile([128, D], F16, tag=f"wk{p}", name=f"wk_sb{p}") for p in range(2)]
            wv_sb = consts.tile([128, 8 * 256], F16, tag="wvm", name="wv_sb")
            wo_sb = [consts.tile([128, D], F16, tag=f"wo{p}", name=f"wo_sb{p}") for p in range(2)]
            for p in range(2):
                nc.sync.dma_start(wq_sb[p][:], wq[p][:])
                nc.sync.dma_start(wk_sb[p][:], wk[p][:])
                nc.sync.dma_start(wo_sb[p][:], wo[p][:])
            nc.sync.dma_start(wv_sb[:], wvm[:])

            # ---- persistent activations ----
            # feature-major Q^T, K^T per pair: rows 0:64 head-even, 64:128 head-odd
            qt = [[persist.tile([128, QC], F16, tag=f"qt{p}_{t}", name=f"qt{p}_{t}")
                   for t in range(NQ)] for p in range(2)]
            kt = [persist.tile([128, T], F16, tag=f"kt{p}", name=f"kt{p}") for p in range(2)]
            # token-major [V_even | 1 | V_odd | 1] per (pair, key-block): [128, 130]
            v2 = [[persist.tile([128, 130], BF16, tag=f"v2_{p}_{b}", name=f"v2_{p}_{b}")
                   for b in range(NKB)] for p in range(2)]
            for p in range(2):
                for b in range(NKB):
                    nc.vector.memset(
                        v2[p][b].rearrange("p (c n) -> p c n", c=2)[:, :, 64:65], 1.0)

            # ---- input DMAs: [128, T] chunks in two halves, d-interleaved so
            # the first token-halves of all 8 chunks land as early as possible
            def alloc_chunks(pfx):
                return [xs.tile([128, T], F16, tag=f"{pfx}{d}", name=f"{pfx}{d}")
                        for d in range(ND)]

            def load_half(ts, x_dram, half):
                c0 = half * (T // 2)
                for d in range(ND):
                    nc.sync.dma_start(ts[d][:, c0:c0 + T // 2],
                                      x_dram[d * 128:(d + 1) * 128, c0:c0 + T // 2])

            xk_t = alloc_chunks("xk")
            xq_t = alloc_chunks("xq")
            xv_t = alloc_chunks("xv")
            load_half(xk_t, xkT, 0)
            load_half(xk_t, xkT, 1)
            load_half(xq_t, xqT, 0)
            load_half(xv_t, xvT, 0)
            load_half(xq_t, xqT, 1)
            load_half(xv_t, xvT, 1)

            # ---- projection helpers ----
            def proj_fm_slice(x_tiles, w_sb, s, evac):
                # feature-major: weights stationary, x moving; psum per pair
                for p in range(2):
                    ps = psP.tile([128, QC], F32, tag="pp", name=f"pj_{id(x_tiles)}_{s}_{p}")
                    for d in range(ND):
                        nc.tensor.matmul(
                            ps[:], w_sb[p][:, d * 128:(d + 1) * 128],
                            x_tiles[d][:, s * QC:(s + 1) * QC],
                            start=(d == 0), stop=(d == ND - 1))
                    evac(p, s, ps)

            def evac_kt(p, s, ps):
                nc.vector.tensor_copy(kt[p][:, s * QC:(s + 1) * QC], ps[:])

            def evac_qt(p, s, ps):
                nc.vector.tensor_copy(qt[p][s][:], ps[:])

            def proj_v_block(b):
                # token-major V: x chunk slice stationary, W_val moving
                ps = psP.tile([128, 256], F32, tag="pp", name=f"pv_{b}")
                for d in range(ND):
                    nc.tensor.matmul(
                        ps[:],
                        xv_t[d][:, b * 128:(b + 1) * 128],
                        wv_sb[:, d * 256:(d + 1) * 256],
                        start=(d == 0), stop=(d == ND - 1))
                for p in range(2):
                    nc.vector.tensor_copy(
                        v2[p][b].rearrange("p (c n) -> p c n", c=2)[:, :, 0:64],
                        ps[:, p * 128:(p + 1) * 128].rearrange(
                            "p (c n) -> p c n", c=2))

            # ---- attention + output projection ----
            ost_live = {}

            def emit_outproj_group(qc, o2t, sub, oc, anchor):
                q0 = qc * QC
                if oc == 0:
                    ost_live[(qc, sub)] = osp.tile(
                        [128, D], F16, tag="os", name=f"os_{qc}_{sub}")
                ost = ost_live[(qc, sub)]
                pp = psP.tile([128, 512], F32, tag="pp", name=f"pp_{qc}_{sub}_{oc}")
                for p in range(2):
                    mm = nc.tensor.matmul(
                        pp[:],
                        o2t[p][:, sub * 128:(sub + 1) * 128],
                        wo_sb[p][:, oc * 512:(oc + 1) * 512],
                        start=(p == 0), stop=(p == 1))
                    if p == 0 and anchor is not None:
                        add_dep_helper(mm.ins, anchor.ins, sync=False,
                                       reason="interleave outproj after S")
                nc.vector.tensor_copy(
                    ost[:, oc * 512:(oc + 1) * 512], pp[:])
                if oc == 1:
                    nc.sync.dma_start(
                        pout[q0 + sub * 128:q0 + (sub + 1) * 128, :],
                        ost[:])
                    del ost_live[(qc, sub)]

            def emit_outproj(qc, o2t, anchor=None):
                for sub in range(4):
                    for oc in range(2):
                        emit_outproj_group(qc, o2t, sub, oc, anchor)

            po_live = {}

            def attn_kb(qc, p, kb, pending):
                # S^T then exp then O^T accumulation for one key block
                k0 = kb * KB
                ps = psS.tile([128, 2 * QC], F32, tag="s", name=f"s_{qc}_{p}_{kb}")
                s_anchor = nc.tensor.matmul(
                    ps[:, 0:QC],
                    kt[p][0:64, k0:k0 + KB],
                    qt[p][qc][0:64, :],
                    start=True, stop=True, tile_position=(0, 0))
                nc.tensor.matmul(
                    ps[:, QC:2 * QC],
                    kt[p][64:128, k0:k0 + KB],
                    qt[p][qc][64:128, :],
                    start=True, stop=True, tile_position=(64, 0))
                at = atp.tile([128, 2 * QC], BF16, tag="at", name=f"at_{qc}_{p}_{kb}")
                nc.scalar.activation(
                    at[:], ps[:], mybir.ActivationFunctionType.Exp)
                po = po_live[(qc, p)]
                for h in range(2):
                    nc.tensor.matmul(
                        po[h][:],
                        v2[p][kb][:, h * 65:h * 65 + 65],
                        at[:, h * QC:(h + 1) * QC],
                        start=(kb == 0), stop=(kb == NKB - 1))
                if p == 1 and pending is not None and kb % 2 == 1:
                    pqc, po2t = pending
                    emit_outproj_group(pqc, po2t, kb // 4, (kb // 2) % 2,
                                       s_anchor)

            def attn_norm(qc, p, o2t):
                # softmax normalization for both heads of pair p
                # (po rows 0:64 = O^T, row 64 = denominator)
                po = po_live.pop((qc, p))
                for h in range(2):
                    ot = smallp.tile([64, QC], F32, tag=f"ot{h}", name=f"ot_{qc}_{p}_{h}")
                    nc.vector.tensor_copy(ot[:], po[h][0:64, :])
                    den = smallp.tile([1, QC], F32, tag=f"den{h}", name=f"den_{qc}_{p}_{h}")
                    nc.vector.tensor_copy(den[:], po[h][64:65, :])
                    r = smallp.tile([1, QC], F32, tag=f"r{h}", name=f"r_{qc}_{p}_{h}")
                    nc.vector.reciprocal_approx_fast(r[:], den[:])
                    rb = smallp.tile([64, QC], F32, tag=f"rb{h}", name=f"rb_{qc}_{p}_{h}")
                    nc.gpsimd.partition_broadcast(rb[:], r[:])
                    nc.gpsimd.tensor_mul(
                        o2t[h * 64:(h + 1) * 64, :],
                        ot[:], rb[:])

            def new_po(qc, p):
                po_live[(qc, p)] = [
                    psO.tile([65, QC], F32, tag=f"o{h}", name=f"po_{qc}_{p}_{h}")
                    for h in range(2)]

            # ---- emission schedule ----
            # K projection first (all slices)
            for s in range(NS):
                proj_fm_slice(xk_t, wk_sb, s, evac_kt)

            # qc=0 attention (pair 0) interleaved with V blocks and Q slices
            o2t_cur = [o2tp.tile([128, QC], F16, tag=f"o2t{p}", name=f"o2t_0_{p}")
                       for p in range(2)]
            new_po(0, 0)
            for g in range(4):
                if g == 0:
                    proj_fm_slice(xq_t, wq_sb, 0, evac_qt)
                for b in range(4 * g, 4 * g + 4):
                    proj_v_block(b)
                if g > 0:
                    proj_fm_slice(xq_t, wq_sb, g, evac_qt)
                for kb in range(4 * g, 4 * g + 4):
                    attn_kb(0, 0, kb, None)
            attn_norm(0, 0, o2t_cur[0])
            new_po(0, 1)
            for kb in range(NKB):
                attn_kb(0, 1, kb, None)
            attn_norm(0, 1, o2t_cur[1])
            pending = (0, o2t_cur)

            # qc=1..3 with previous qc's outproj interleaved into pair 1
            for qc in range(1, NQ):
                o2t_cur = [o2tp.tile([128, QC], F16, tag=f"o2t{p}", name=f"o2t_{qc}_{p}")
                           for p in range(2)]
                for p in range(2):
                    new_po(qc, p)
                    for kb in range(NKB):
                        attn_kb(qc, p, kb, pending if p == 1 else None)
                    attn_norm(qc, p, o2t_cur[p])
                pending = (qc, o2t_cur)
            emit_outproj(*pending)

    nc.compile()
    nc.m = get_hw_module(nc.m)
    return nc


def _pack_w(w_pair):
    # w_pair: [2, 1024, 64] -> [1024, 128] -> chunk-major [128, 8*128]
    w = np.concatenate([w_pair[0], w_pair[1]], axis=1)          # [1024, 128]
    return np.ascontiguousarray(
        w.reshape(ND, 128, 128).transpose(1, 0, 2).reshape(128, D))


def _pack_wv(w4):
    # w4: [4, 1024, 64] -> [1024, 256] -> chunk-major [128, 8*256]
    w = np.concatenate([w4[h] for h in range(4)], axis=1)       # [1024, 256]
    return np.ascontiguousarray(
        w.reshape(ND, 128, 256).transpose(1, 0, 2).reshape(128, ND * 256))


def _pack_wo(wo_pair):
    # wo_pair: [2, 64, 1024] -> [128, 1024]
    return np.ascontiguousarray(np.concatenate([wo_pair[0], wo_pair[1]], axis=0))


def kernel(q, k, v, W_query, W_key, W_val, W_out, _trace=False):
    q = np.asarray(q, dtype=np.float32)
    k = np.asarray(k, dtype=np.float32)
    v = np.asarray(v, dtype=np.float32)
    W_query = np.asarray(W_query, dtype=np.float32)
    W_key = np.asarray(W_key, dtype=np.float32)
    W_val = np.asarray(W_val, dtype=np.float32)
    W_out = np.asarray(W_out, dtype=np.float32)

    if "nc" not in _CACHE:
        _CACHE["nc"] = _build()
    nc = _CACHE["nc"]

    norm = 1.0 / np.sqrt(E)
    xT = {}
    for b in range(2):
        xT[("q", b)] = np.ascontiguousarray(q[b].T).astype(np.float16)
        xT[("k", b)] = np.ascontiguousarray(k[b].T).astype(np.float16)
        xT[("v", b)] = np.ascontiguousarray(v[b].T).astype(np.float16)

    in_maps = []
    for c in range(N_CORES):
        b, g = c // 4, c % 4
        hs = [4 * g, 4 * g + 1, 4 * g + 2, 4 * g + 3]
        m = {
            "xqT": xT[("q", b)], "xkT": xT[("k", b)], "xvT": xT[("v", b)],
            "wvm": _pack_wv(W_val[hs]).astype(np.float16),
        }
        for p in range(2):
            hp = hs[2 * p:2 * p + 2]
            m[f"wq{p}"] = _pack_w(W_query[hp] * norm).astype(np.float16)
            m[f"wk{p}"] = _pack_w(W_key[hp]).astype(np.float16)
            m[f"wo{p}"] = _pack_wo(W_out[hp]).astype(np.float16)
        in_maps.append(m)

    res = run_bass_kernel_spmd(nc, in_maps, list(range(N_CORES)),
                               trace=_trace)
    parts = [res.results[c]["pout"].astype(np.float32) for c in range(N_CORES)]
    out = np.stack([
        parts[0] + parts[1] + parts[2] + parts[3],
        parts[4] + parts[5] + parts[6] + parts[7],
    ]).astype(np.float32)
    if _trace:
        _CACHE["last_result"] = res
    return out


# revision 25
# speedup vs baseline: 1.2068x; 1.1754x over previous
"""Trainium2 Bass kernel for 16-head MHA (B=2, S=2048, D=1024, E=64).

Sharding: 8 cores = 2 batches x 4 head-groups. Each core computes 4 heads
(2 pairs of 2) for one batch and returns a partial output [2048, 1024]
(sum of its 4 heads' contributions after the output projection). Host sums
the 4 partials per batch.

Per-core pipeline (all matmuls on PE, fp32 PSUM accumulation):
  - K/Q projections feature-major (weights stationary, x moving)
  - V projection token-major directly on the PE (x chunk stationary,
    W_val moving) -- avoids DMA transposes entirely
  - S^T = K Q^T per head pair, two heads row-packed in the 128x128 array
  - A^T = exp(S^T) on ACT (scale folded into W_query on host); ACT does
    ONLY exp -- all psum evacuations go through DVE
  - O^T accumulation with fused row-sum via a ones column in the V tiles
  - softmax normalization: DVE reciprocal_approx_fast + GPSIMD
    partition-broadcast + DVE multiply (writes fp16 O^T)
  - output projection (fp16) accumulating both pairs, fp16 partials out
  - phase 1 is software-pipelined into attention: slice-ordered DMAs,
    K proj first, V-blocks + Q-slices interleaved with attention qc=0
"""

import sys

sys.path.insert(0, "/opt/trn_rl_repo")

import numpy as np

import concourse.bass as bass
import concourse.bacc as bacc
import concourse.mybir as mybir
from concourse import tile
from concourse.tile_rust import add_dep_helper
from concourse.bass_interp import get_hw_module
from concourse.bass_utils import run_bass_kernel_spmd

F16 = mybir.dt.float16
F32 = mybir.dt.float32
BF16 = mybir.dt.bfloat16
I16 = mybir.dt.int16

# Schraudolph exp: bf16 bits = round(x * 128/ln2 + B); B tuned for zero mean
# relative error so softmax numerator/denominator biases cancel
SCHRAUD_A = float(np.float32(128.0 / np.log(2.0)))
SCHRAUD_B = 16250.0

N_CORES = 8
T = 2048          # tokens per core (one batch)
D = 1024          # model dim
E = 64            # head dim
QC = 512          # query chunk
NQ = T // QC      # 4 query chunks
KB = 128          # key block
NKB = T // KB     # 16 key blocks
ND = D // 128     # 8 contraction chunks for projections
NS = 4            # token slices (512 each)

_CACHE = {}


def _build():
    nc = bacc.Bacc("TRN2", target_bir_lowering=False, debug=False,
                   num_devices=N_CORES)

    xqT = nc.dram_tensor("xqT", [D, T], F16, kind="ExternalInput").ap()
    xkT = nc.dram_tensor("xkT", [D, T], F16, kind="ExternalInput").ap()
    xvT = nc.dram_tensor("xvT", [D, T], F16, kind="ExternalInput").ap()
    # per-pair packed weights, layout [128, 8*128]: chunk d at cols d*128
    wq = [nc.dram_tensor(f"wq{p}", [128, D], F16, kind="ExternalInput").ap()
          for p in range(2)]
    wk = [nc.dram_tensor(f"wk{p}", [128, D], F16, kind="ExternalInput").ap()
          for p in range(2)]
    # V weights as moving operand: [128, 8*256]; chunk d at cols d*256,
    # within a chunk cols h*64:(h+1)*64 = head h features
    wvm = nc.dram_tensor("wvm", [128, 8 * 256], F16, kind="ExternalInput").ap()
    wo = [nc.dram_tensor(f"wo{p}", [128, D], F16, kind="ExternalInput").ap()
          for p in range(2)]
    pout = nc.dram_tensor("pout", [T, D], F16, kind="ExternalOutput").ap()

    with tile.TileContext(nc) as tc:
        with (
            tc.tile_pool(name="consts", bufs=1) as consts,
            tc.tile_pool(name="persist", bufs=1) as persist,
            tc.tile_pool(name="xs", bufs=1) as xs,
            tc.tile_pool(name="at", bufs=5) as atp,
            tc.tile_pool(name="o2t", bufs=2) as o2tp,
            tc.tile_pool(name="os", bufs=3) as osp,
            tc.tile_pool(name="small", bufs=2) as smallp,
            tc.tile_pool(name="psS", bufs=2, space="PSUM") as psS,
            tc.tile_pool(name="psO", bufs=1, space="PSUM") as psO,
            tc.tile_pool(name="psP", bufs=2, space="PSUM") as psP,
        ):
            # ---- weights ----
            wq_sb = [consts.tile([128, D], F16, tag=f"wq{p}", name=f"wq_sb{p}") for p in range(2)]
            wk_sb = [consts.tile([128, D], F16, tag=f"wk{p}", name=f"wk_sb{p}") for p in range(2)]
            wv_sb = consts.tile([128, 8 * 256], F16, tag="wvm", name="wv_sb")
            wo_sb = [consts.tile([128, D], F16, tag=f"wo{p}", name=f"wo_sb{p}") for p in range(2)]
            for p in range(2):
                nc.sync.dma_start(wq_sb[p][:], wq[p][:])
                nc.sync.dma_start(wk_sb[p][:], wk[p][:])
                nc.sync.dma_start(wo_sb[p][:], wo[p][:])
            nc.sync.dma_start(wv_sb[:], wvm[:])

            # ---- persistent activations ----
            # feature-major Q^T, K^T per pair: rows 0:64 head-even, 64:128 head-odd
            qt = [[persist.tile([128, QC], F16, tag=f"qt{p}_{t}", name=f"qt{p}_{t}")
                   for t in range(NQ)] for p in range(2)]
            kt = [persist.tile([128, T], F16, tag=f"kt{p}", name=f"kt{p}") for p in range(2)]
            # token-major [V_even | 1 | V_odd | 1] per (pair, key-block): [128, 130]
            v2 = [[persist.tile([128, 130], BF16, tag=f"v2_{p}_{b}", name=f"v2_{p}_{b}")
                   for b in range(NKB)] for p in range(2)]
            for p in range(2):
                for b in range(NKB):
                    nc.vector.memset(
                        v2[p][b].rearrange("p (c n) -> p c n", c=2)[:, :, 64:65], 1.0)

            # ---- input DMAs: full [128, T] chunks, ordered k, q, v ----
            def load_chunks(x_dram, pfx):
                ts = []
                for d in range(ND):
                    t = xs.tile([128, T], F16, tag=f"{pfx}{d}", name=f"{pfx}{d}")
                    nc.sync.dma_start(t[:], x_dram[d * 128:(d + 1) * 128, :])
                    ts.append(t)
                return ts

            xk_t = load_chunks(xkT, "xk")
            xq_t = load_chunks(xqT, "xq")
            xv_t = load_chunks(xvT, "xv")

            # ---- HAM warmup: junk matmuls on the (early-arriving) weights
            # fill the input-DMA hole and flip the PE clock gate to 8/8
            # before the real projections start
            warm = psP.tile([128, QC], F32, tag="pp", name="warm")
            for i in range(24):
                nc.tensor.matmul(warm[:], wq_sb[0][:, 0:128],
                                 wq_sb[0][:, 0:QC], start=True, stop=True)

            # ---- projection helpers ----
            def proj_fm_slice(x_tiles, w_sb, s, evac):
                # feature-major: weights stationary, x moving; psum per pair
                for p in range(2):
                    ps = psP.tile([128, QC], F32, tag="pp", name=f"pj_{id(x_tiles)}_{s}_{p}")
                    for d in range(ND):
                        nc.tensor.matmul(
                            ps[:], w_sb[p][:, d * 128:(d + 1) * 128],
                            x_tiles[d][:, s * QC:(s + 1) * QC],
                            start=(d == 0), stop=(d == ND - 1))
                    evac(p, s, ps)

            def evac_kt(p, s, ps):
                nc.vector.tensor_copy(kt[p][:, s * QC:(s + 1) * QC], ps[:])

            def evac_qt(p, s, ps):
                nc.vector.tensor_copy(qt[p][s][:], ps[:])

            def proj_v_block(b):
                # token-major V: x chunk slice stationary, W_val moving
                ps = psP.tile([128, 256], F32, tag="pp", name=f"pv_{b}")
                for d in range(ND):
                    nc.tensor.matmul(
                        ps[:],
                        xv_t[d][:, b * 128:(b + 1) * 128],
                        wv_sb[:, d * 256:(d + 1) * 256],
                        start=(d == 0), stop=(d == ND - 1))
                for p in range(2):
                    nc.vector.tensor_copy(
                        v2[p][b].rearrange("p (c n) -> p c n", c=2)[:, :, 0:64],
                        ps[:, p * 128:(p + 1) * 128].rearrange(
                            "p (c n) -> p c n", c=2))

            # ---- attention + output projection ----
            ost_live = {}

            def emit_outproj_group(qc, o2t, sub, oc, anchor):
                q0 = qc * QC
                if oc == 0:
                    ost_live[(qc, sub)] = osp.tile(
                        [128, D], F16, tag="os", name=f"os_{qc}_{sub}")
                ost = ost_live[(qc, sub)]
                pp = psP.tile([128, 512], F32, tag="pp", name=f"pp_{qc}_{sub}_{oc}")
                for p in range(2):
                    mm = nc.tensor.matmul(
                        pp[:],
                        o2t[p][:, sub * 128:(sub + 1) * 128],
                        wo_sb[p][:, oc * 512:(oc + 1) * 512],
                        start=(p == 0), stop=(p == 1))
                    if p == 0 and anchor is not None:
                        add_dep_helper(mm.ins, anchor.ins, sync=False,
                                       reason="interleave outproj after S")
                nc.vector.tensor_copy(
                    ost[:, oc * 512:(oc + 1) * 512], pp[:])
                if oc == 1:
                    nc.sync.dma_start(
                        pout[q0 + sub * 128:q0 + (sub + 1) * 128, :],
                        ost[:])
                    del ost_live[(qc, sub)]

            def emit_outproj(qc, o2t, anchor=None):
                for sub in range(4):
                    for oc in range(2):
                        emit_outproj_group(qc, o2t, sub, oc, anchor)

            po_live = {}

            def attn_kb(qc, p, kb, pending):
                # S^T then exp then O^T accumulation for one key block
                k0 = kb * KB
                ps = psS.tile([128, 2 * QC], F32, tag="s", name=f"s_{qc}_{p}_{kb}")
                s_anchor = nc.tensor.matmul(
                    ps[:, 0:QC],
                    kt[p][0:64, k0:k0 + KB],
                    qt[p][qc][0:64, :],
                    start=True, stop=True, tile_position=(0, 0))
                nc.tensor.matmul(
                    ps[:, QC:2 * QC],
                    kt[p][64:128, k0:k0 + KB],
                    qt[p][qc][64:128, :],
                    start=True, stop=True, tile_position=(64, 0))
                at = atp.tile([128, 2 * QC], BF16, tag="at", name=f"at_{qc}_{p}_{kb}")
                nc.scalar.activation(
                    at[:], ps[:], mybir.ActivationFunctionType.Exp)
                po = po_live[(qc, p)]
                for h in range(2):
                    nc.tensor.matmul(
                        po[h][:],
                        v2[p][kb][:, h * 65:h * 65 + 65],
                        at[:, h * QC:(h + 1) * QC],
                        start=(kb == 0), stop=(kb == NKB - 1))
                if p == 1 and pending is not None and kb % 2 == 1:
                    pqc, po2t = pending
                    emit_outproj_group(pqc, po2t, kb // 4, (kb // 2) % 2,
                                       s_anchor)

            def attn_norm(qc, p, o2t):
                # softmax normalization for both heads of pair p
                # (po rows 0:64 = O^T, row 64 = denominator)
                po = po_live.pop((qc, p))
                ots, rbs = [], []
                for h in range(2):
                    # denominator chain first -- it has the longest latency
                    # (recip -> gpsimd broadcast) before the final multiply
                    den = smallp.tile([1, QC], F32, tag=f"den{h}", name=f"den_{qc}_{p}_{h}")
                    nc.vector.tensor_copy(den[:], po[h][64:65, :])
                    r = smallp.tile([1, QC], F32, tag=f"r{h}", name=f"r_{qc}_{p}_{h}")
                    nc.vector.reciprocal_approx_fast(r[:], den[:])
                    rb = smallp.tile([64, QC], F32, tag=f"rb{h}", name=f"rb_{qc}_{p}_{h}")
                    nc.gpsimd.partition_broadcast(rb[:], r[:])
                    rbs.append(rb)
                for h in range(2):
                    ot = smallp.tile([64, QC], F32, tag=f"ot{h}", name=f"ot_{qc}_{p}_{h}")
                    nc.vector.tensor_copy(ot[:], po[h][0:64, :])
                    ots.append(ot)
                for h in range(2):
                    nc.vector.tensor_mul(
                        o2t[h * 64:(h + 1) * 64, :],
                        ots[h][:], rbs[h][:])

            def new_po(qc, p):
                po_live[(qc, p)] = [
                    psO.tile([65, QC], F32, tag=f"o{h}", name=f"po_{qc}_{p}_{h}")
                    for h in range(2)]

            # ---- emission schedule ----
            # K projection first (all slices)
            for s in range(NS):
                proj_fm_slice(xk_t, wk_sb, s, evac_kt)

            # qc=0 attention (pair 0) interleaved with V blocks and Q slices
            o2t_cur = [o2tp.tile([128, QC], F16, tag=f"o2t{p}", name=f"o2t_0_{p}")
                       for p in range(2)]
            new_po(0, 0)
            for g in range(4):
                if g == 0:
                    proj_fm_slice(xq_t, wq_sb, 0, evac_qt)
                for b in range(4 * g, 4 * g + 4):
                    proj_v_block(b)
                if g > 0:
                    proj_fm_slice(xq_t, wq_sb, g, evac_qt)
                for kb in range(4 * g, 4 * g + 4):
                    attn_kb(0, 0, kb, None)
            attn_norm(0, 0, o2t_cur[0])
            new_po(0, 1)
            for kb in range(NKB):
                attn_kb(0, 1, kb, None)
            attn_norm(0, 1, o2t_cur[1])
            pending = (0, o2t_cur)

            # qc=1..3 with previous qc's outproj interleaved into pair 1
            for qc in range(1, NQ):
                o2t_cur = [o2tp.tile([128, QC], F16, tag=f"o2t{p}", name=f"o2t_{qc}_{p}")
                           for p in range(2)]
                for p in range(2):
                    new_po(qc, p)
                    for kb in range(NKB):
                        attn_kb(qc, p, kb, pending if p == 1 else None)
                    attn_norm(qc, p, o2t_cur[p])
                pending = (qc, o2t_cur)
            emit_outproj(*pending)

    nc.compile()
    nc.m = get_hw_module(nc.m)
    return nc


def _pack_w(w_pair):
    # w_pair: [2, 1024, 64] -> [1024, 128] -> chunk-major [128, 8*128]
    w = np.concatenate([w_pair[0], w_pair[1]], axis=1)          # [1024, 128]
    return np.ascontiguousarray(
        w.reshape(ND, 128, 128).transpose(1, 0, 2).reshape(128, D))


def _pack_wv(w4):
    # w4: [4, 1024, 64] -> [1024, 256] -> chunk-major [128, 8*256]
    w = np.concatenate([w4[h] for h in range(4)], axis=1)       # [1024, 256]
    return np.ascontiguousarray(
        w.reshape(ND, 128, 256).transpose(1, 0, 2).reshape(128, ND * 256))


def _pack_wo(wo_pair):
    # wo_pair: [2, 64, 1024] -> [128, 1024]
    return np.ascontiguousarray(np.concatenate([wo_pair[0], wo_pair[1]], axis=0))


def kernel(q, k, v, W_query, W_key, W_val, W_out, _trace=False):
    q = np.asarray(q, dtype=np.float32)
    k = np.asarray(k, dtype=np.float32)
    v = np.asarray(v, dtype=np.float32)
    W_query = np.asarray(W_query, dtype=np.float32)
    W_key = np.asarray(W_key, dtype=np.float32)
    W_val = np.asarray(W_val, dtype=np.float32)
    W_out = np.asarray(W_out, dtype=np.float32)

    if "nc" not in _CACHE:
        _CACHE["nc"] = _build()
    nc = _CACHE["nc"]

    norm = 1.0 / np.sqrt(E)
    xT = {}
    for b in range(2):
        xT[("q", b)] = np.ascontiguousarray(q[b].T).astype(np.float16)
        xT[("k", b)] = np.ascontiguousarray(k[b].T).astype(np.float16)
        xT[("v", b)] = np.ascontiguousarray(v[b].T).astype(np.float16)

    in_maps = []
    for c in range(N_CORES):
        b, g = c // 4, c % 4
        hs = [4 * g, 4 * g + 1, 4 * g + 2, 4 * g + 3]
        m = {
            "xqT": xT[("q", b)], "xkT": xT[("k", b)], "xvT": xT[("v", b)],
            "wvm": _pack_wv(W_val[hs]).astype(np.float16),
        }
        for p in range(2):
            hp = hs[2 * p:2 * p + 2]
            m[f"wq{p}"] = _pack_w(W_query[hp] * norm).astype(np.float16)
            m[f"wk{p}"] = _pack_w(W_key[hp]).astype(np.float16)
            m[f"wo{p}"] = _pack_wo(W_out[hp]).astype(np.float16)
        in_maps.append(m)

    res = run_bass_kernel_spmd(nc, in_maps, list(range(N_CORES)),
                               trace=_trace)
    parts = [res.results[c]["pout"].astype(np.float32) for c in range(N_CORES)]
    out = np.stack([
        parts[0] + parts[1] + parts[2] + parts[3],
        parts[4] + parts[5] + parts[6] + parts[7],
    ]).astype(np.float32)
    if _trace:
        _CACHE["last_result"] = res
    return out


# revision 32
# speedup vs baseline: 1.3941x; 1.1552x over previous
"""Trainium2 Bass kernel for 16-head MHA (B=2, S=2048, D=1024, E=64).

Sharding: 8 cores = 2 batches x 4 head-groups. Each core computes 4 heads
(2 pairs of 2) for one batch and returns a partial output [2048, 1024]
(sum of its 4 heads' contributions after the output projection). Host sums
the 4 partials per batch.

Per-core pipeline (all matmuls on PE, fp32 PSUM accumulation):
  - K/Q projections feature-major (weights stationary, x moving)
  - V projection token-major directly on the PE (x chunk stationary,
    W_val moving) -- avoids DMA transposes entirely
  - S^T = K Q^T per head pair, two heads row-packed in the 128x128 array
  - A^T = exp(S^T) on ACT (scale folded into W_query on host); ACT does
    ONLY exp -- all psum evacuations go through DVE
  - O^T accumulation with fused row-sum via a ones column in the V tiles
  - softmax normalization: DVE reciprocal_approx_fast + GPSIMD
    partition-broadcast + DVE multiply (writes fp16 O^T)
  - output projection (fp16) accumulating both pairs, fp16 partials out
  - phase 1 is software-pipelined into attention: slice-ordered DMAs,
    K proj first, V-blocks + Q-slices interleaved with attention qc=0
"""

import sys

sys.path.insert(0, "/opt/trn_rl_repo")

import numpy as np

import concourse.bass as bass
import concourse.bacc as bacc
import concourse.mybir as mybir
from concourse import tile
from concourse.tile_rust import add_dep_helper
from concourse.bass_interp import get_hw_module
from concourse.bass_utils import run_bass_kernel_spmd

F16 = mybir.dt.float16
F32 = mybir.dt.float32
BF16 = mybir.dt.bfloat16
I16 = mybir.dt.int16

# Schraudolph exp: bf16 bits = round(x * 128/ln2 + B); B tuned for zero mean
# relative error so softmax numerator/denominator biases cancel
SCHRAUD_A = float(np.float32(128.0 / np.log(2.0)))
SCHRAUD_B = 16250.0

N_CORES = 8
T = 2048          # tokens per core (one batch)
D = 1024          # model dim
E = 64            # head dim
QC = 512          # query chunk
NQ = T // QC      # 4 query chunks
KB = 128          # key block
NKB = T // KB     # 16 key blocks
ND = D // 128     # 8 contraction chunks for projections
NS = 4            # token slices (512 each)

_CACHE = {}


def _build():
    nc = bacc.Bacc("TRN2", target_bir_lowering=False, debug=False,
                   num_devices=N_CORES)

    xqT = nc.dram_tensor("xqT", [D, T], F16, kind="ExternalInput").ap()
    xkT = nc.dram_tensor("xkT", [D, T], F16, kind="ExternalInput").ap()
    xvT = nc.dram_tensor("xvT", [D, T], F16, kind="ExternalInput").ap()
    # packed weights, both pairs side by side: pair p at cols p*D, within a
    # pair chunk d at cols d*128
    wq = nc.dram_tensor("wq", [128, 2 * D], F16, kind="ExternalInput").ap()
    wk = nc.dram_tensor("wk", [128, 2 * D], F16, kind="ExternalInput").ap()
    # V weights as moving operand: [128, 8*256]; chunk d at cols d*256,
    # within a chunk cols h*64:(h+1)*64 = head h features
    wvm = nc.dram_tensor("wvm", [128, 8 * 256], F16, kind="ExternalInput").ap()
    wo = nc.dram_tensor("wo", [128, 2 * D], F16, kind="ExternalInput").ap()
    pout = nc.dram_tensor("pout", [T, D], F16, kind="ExternalOutput").ap()

    with tile.TileContext(nc) as tc:
        with (
            tc.tile_pool(name="consts", bufs=1) as consts,
            tc.tile_pool(name="persist", bufs=1) as persist,
            tc.tile_pool(name="xs", bufs=1) as xs,
            tc.tile_pool(name="at", bufs=5) as atp,
            tc.tile_pool(name="o2t", bufs=2) as o2tp,
            tc.tile_pool(name="os", bufs=3) as osp,
            tc.tile_pool(name="small", bufs=2) as smallp,
            tc.tile_pool(name="psS", bufs=2, space="PSUM") as psS,
            tc.tile_pool(name="psO", bufs=1, space="PSUM") as psO,
            tc.tile_pool(name="psP", bufs=2, space="PSUM") as psP,
        ):
            # ---- weights (descriptor gen spread across idle engine queues) ----
            wq_sb = consts.tile([128, 2 * D], F16, tag="wq", name="wq_sb")
            wk_sb = consts.tile([128, 2 * D], F16, tag="wk", name="wk_sb")
            wv_sb = consts.tile([128, 8 * 256], F16, tag="wvm", name="wv_sb")
            wo_sb = consts.tile([128, 2 * D], F16, tag="wo", name="wo_sb")
            nc.sync.dma_start(wq_sb[:], wq[:])
            nc.gpsimd.dma_start(wk_sb[:], wk[:])
            nc.scalar.dma_start(wv_sb[:], wvm[:])
            nc.scalar.dma_start(wo_sb[:], wo[:])

            # ---- persistent activations ----
            # feature-major Q^T, K^T per pair: rows 0:64 head-even, 64:128 head-odd
            qt = [[persist.tile([128, QC], F16, tag=f"qt{p}_{t}", name=f"qt{p}_{t}")
                   for t in range(NQ)] for p in range(2)]
            kt = [persist.tile([128, T], F16, tag=f"kt{p}", name=f"kt{p}") for p in range(2)]
            # token-major [V_even | 1 | V_odd | 1] per (pair, key-block): [128, 130]
            v2 = [[persist.tile([128, 130], BF16, tag=f"v2_{p}_{b}", name=f"v2_{p}_{b}")
                   for b in range(NKB)] for p in range(2)]
            for p in range(2):
                for b in range(NKB):
                    nc.vector.memset(
                        v2[p][b].rearrange("p (c n) -> p c n", c=2)[:, :, 64:65], 1.0)

            # ---- input DMAs: full [128, T] chunks, ordered k, q, v; two
            # engine queues alternate so descriptor gen is not serial
            def load_chunks(x_dram, pfx):
                ts = []
                for d in range(ND):
                    t = xs.tile([128, T], F16, tag=f"{pfx}{d}", name=f"{pfx}{d}")
                    eng = nc.sync if d % 2 == 0 else nc.gpsimd
                    eng.dma_start(t[:], x_dram[d * 128:(d + 1) * 128, :])
                    ts.append(t)
                return ts

            xk_t = load_chunks(xkT, "xk")
            xq_t = load_chunks(xqT, "xq")
            xv_t = load_chunks(xvT, "xv")

            # ---- HAM warmup: junk matmuls on the (early-arriving) weights
            # fill the input-DMA hole and flip the PE clock gate to 8/8
            # before the real projections start
            warm = psP.tile([128, QC], F32, tag="pp", name="warm")
            for i in range(14):
                nc.tensor.matmul(warm[:], wq_sb[:, 0:128],
                                 wq_sb[:, 0:QC], start=True, stop=True)

            # ---- projection helpers ----
            def proj_fm_slice(x_tiles, w_sb, s, evac):
                # feature-major: weights stationary, x moving; psum per pair
                for p in range(2):
                    ps = psP.tile([128, QC], F32, tag="pp", name=f"pj_{id(x_tiles)}_{s}_{p}")
                    for d in range(ND):
                        nc.tensor.matmul(
                            ps[:], w_sb[:, p * D + d * 128:p * D + (d + 1) * 128],
                            x_tiles[d][:, s * QC:(s + 1) * QC],
                            start=(d == 0), stop=(d == ND - 1))
                    evac(p, s, ps)

            def evac_kt(p, s, ps):
                nc.vector.tensor_copy(kt[p][:, s * QC:(s + 1) * QC], ps[:])

            def evac_qt(p, s, ps):
                nc.vector.tensor_copy(qt[p][s][:], ps[:])

            def proj_v_block(b):
                # token-major V: x chunk slice stationary, W_val moving
                ps = psP.tile([128, 256], F32, tag="pp", name=f"pv_{b}")
                for d in range(ND):
                    nc.tensor.matmul(
                        ps[:],
                        xv_t[d][:, b * 128:(b + 1) * 128],
                        wv_sb[:, d * 256:(d + 1) * 256],
                        start=(d == 0), stop=(d == ND - 1))
                for p in range(2):
                    nc.vector.tensor_copy(
                        v2[p][b].rearrange("p (c n) -> p c n", c=2)[:, :, 0:64],
                        ps[:, p * 128:(p + 1) * 128].rearrange(
                            "p (c n) -> p c n", c=2))

            # ---- attention + output projection ----
            ost_live = {}

            def emit_outproj_group(qc, o2t, sub, oc, anchor):
                q0 = qc * QC
                if oc == 0:
                    ost_live[(qc, sub)] = osp.tile(
                        [128, D], F16, tag="os", name=f"os_{qc}_{sub}")
                ost = ost_live[(qc, sub)]
                pp = psP.tile([128, 512], F32, tag="pp", name=f"pp_{qc}_{sub}_{oc}")
                for p in range(2):
                    mm = nc.tensor.matmul(
                        pp[:],
                        o2t[p][:, sub * 128:(sub + 1) * 128],
                        wo_sb[:, p * D + oc * 512:p * D + (oc + 1) * 512],
                        start=(p == 0), stop=(p == 1))
                    if p == 0 and anchor is not None:
                        add_dep_helper(mm.ins, anchor.ins, sync=False,
                                       reason="interleave outproj after S")
                nc.vector.tensor_copy(
                    ost[:, oc * 512:(oc + 1) * 512], pp[:])
                if oc == 1:
                    nc.sync.dma_start(
                        pout[q0 + sub * 128:q0 + (sub + 1) * 128, :],
                        ost[:])
                    del ost_live[(qc, sub)]

            def emit_outproj(qc, o2t, anchor=None):
                for sub in range(4):
                    for oc in range(2):
                        emit_outproj_group(qc, o2t, sub, oc, anchor)

            po_live = {}

            def attn_kb(qc, p, kb, pending):
                # S^T then exp then O^T accumulation for one key block
                k0 = kb * KB
                ps = psS.tile([128, 2 * QC], F32, tag="s", name=f"s_{qc}_{p}_{kb}")
                s_anchor = nc.tensor.matmul(
                    ps[:, 0:QC],
                    kt[p][0:64, k0:k0 + KB],
                    qt[p][qc][0:64, :],
                    start=True, stop=True, tile_position=(0, 0))
                nc.tensor.matmul(
                    ps[:, QC:2 * QC],
                    kt[p][64:128, k0:k0 + KB],
                    qt[p][qc][64:128, :],
                    start=True, stop=True, tile_position=(64, 0))
                at = atp.tile([128, 2 * QC], BF16, tag="at", name=f"at_{qc}_{p}_{kb}")
                nc.scalar.activation(
                    at[:], ps[:], mybir.ActivationFunctionType.Exp)
                po = po_live[(qc, p)]
                for h in range(2):
                    nc.tensor.matmul(
                        po[h][:],
                        v2[p][kb][:, h * 65:h * 65 + 65],
                        at[:, h * QC:(h + 1) * QC],
                        start=(kb == 0), stop=(kb == NKB - 1))
                if p == 1 and pending is not None and kb % 2 == 1:
                    pqc, po2t = pending
                    emit_outproj_group(pqc, po2t, kb // 4, (kb // 2) % 2,
                                       s_anchor)

            def attn_norm(qc, p, o2t):
                # softmax normalization for both heads of pair p
                # (po rows 0:64 = O^T, row 64 = denominator)
                po = po_live.pop((qc, p))
                ots, rbs = [], []
                for h in range(2):
                    # denominator chain first -- it has the longest latency
                    # (recip -> gpsimd broadcast) before the final multiply
                    den = smallp.tile([1, QC], F32, tag=f"den{h}", name=f"den_{qc}_{p}_{h}")
                    nc.vector.tensor_copy(den[:], po[h][64:65, :])
                    r = smallp.tile([1, QC], F32, tag=f"r{h}", name=f"r_{qc}_{p}_{h}")
                    nc.vector.reciprocal_approx_fast(r[:], den[:])
                    rb = smallp.tile([64, QC], F32, tag=f"rb{h}", name=f"rb_{qc}_{p}_{h}")
                    nc.gpsimd.partition_broadcast(rb[:], r[:])
                    rbs.append(rb)
                for h in range(2):
                    ot = smallp.tile([64, QC], F32, tag=f"ot{h}", name=f"ot_{qc}_{p}_{h}")
                    nc.vector.tensor_copy(ot[:], po[h][0:64, :])
                    ots.append(ot)
                for h in range(2):
                    nc.vector.tensor_mul(
                        o2t[h * 64:(h + 1) * 64, :],
                        ots[h][:], rbs[h][:])

            def new_po(qc, p):
                po_live[(qc, p)] = [
                    psO.tile([65, QC], F32, tag=f"o{h}", name=f"po_{qc}_{p}_{h}")
                    for h in range(2)]

            # ---- emission schedule ----
            # K projection first (all slices)
            for s in range(NS):
                proj_fm_slice(xk_t, wk_sb, s, evac_kt)

            # qc=0 attention (pair 0) interleaved with V blocks and Q slices
            o2t_cur = [o2tp.tile([128, QC], F16, tag=f"o2t{p}", name=f"o2t_0_{p}")
                       for p in range(2)]
            new_po(0, 0)
            for g in range(4):
                if g == 0:
                    proj_fm_slice(xq_t, wq_sb, 0, evac_qt)
                for b in range(4 * g, 4 * g + 4):
                    proj_v_block(b)
                if g > 0:
                    proj_fm_slice(xq_t, wq_sb, g, evac_qt)
                for kb in range(4 * g, 4 * g + 4):
                    attn_kb(0, 0, kb, None)
            attn_norm(0, 0, o2t_cur[0])
            new_po(0, 1)
            for kb in range(NKB):
                attn_kb(0, 1, kb, None)
            attn_norm(0, 1, o2t_cur[1])
            pending = (0, o2t_cur)

            # qc=1..3 with previous qc's outproj interleaved into pair 1
            for qc in range(1, NQ):
                o2t_cur = [o2tp.tile([128, QC], F16, tag=f"o2t{p}", name=f"o2t_{qc}_{p}")
                           for p in range(2)]
                for p in range(2):
                    new_po(qc, p)
                    for kb in range(NKB):
                        attn_kb(qc, p, kb, pending if p == 1 else None)
                    attn_norm(qc, p, o2t_cur[p])
                pending = (qc, o2t_cur)
            emit_outproj(*pending)

    nc.compile()
    nc.m = get_hw_module(nc.m)
    return nc


def _pack_w(w_pair):
    # w_pair: [2, 1024, 64] -> [1024, 128] -> chunk-major [128, 8*128]
    w = np.concatenate([w_pair[0], w_pair[1]], axis=1)          # [1024, 128]
    return np.ascontiguousarray(
        w.reshape(ND, 128, 128).transpose(1, 0, 2).reshape(128, D))


def _pack_wv(w4):
    # w4: [4, 1024, 64] -> [1024, 256] -> chunk-major [128, 8*256]
    w = np.concatenate([w4[h] for h in range(4)], axis=1)       # [1024, 256]
    return np.ascontiguousarray(
        w.reshape(ND, 128, 256).transpose(1, 0, 2).reshape(128, ND * 256))


def _pack_wo(wo_pair):
    # wo_pair: [2, 64, 1024] -> [128, 1024]
    return np.ascontiguousarray(np.concatenate([wo_pair[0], wo_pair[1]], axis=0))


def kernel(q, k, v, W_query, W_key, W_val, W_out, _trace=False):
    q = np.asarray(q, dtype=np.float32)
    k = np.asarray(k, dtype=np.float32)
    v = np.asarray(v, dtype=np.float32)
    W_query = np.asarray(W_query, dtype=np.float32)
    W_key = np.asarray(W_key, dtype=np.float32)
    W_val = np.asarray(W_val, dtype=np.float32)
    W_out = np.asarray(W_out, dtype=np.float32)

    if "nc" not in _CACHE:
        _CACHE["nc"] = _build()
    nc = _CACHE["nc"]

    norm = 1.0 / np.sqrt(E)
    xT = {}
    for b in range(2):
        xT[("q", b)] = np.ascontiguousarray(q[b].T).astype(np.float16)
        xT[("k", b)] = np.ascontiguousarray(k[b].T).astype(np.float16)
        xT[("v", b)] = np.ascontiguousarray(v[b].T).astype(np.float16)

    in_maps = []
    for c in range(N_CORES):
        b, g = c // 4, c % 4
        hs = [4 * g, 4 * g + 1, 4 * g + 2, 4 * g + 3]
        m = {
            "xqT": xT[("q", b)], "xkT": xT[("k", b)], "xvT": xT[("v", b)],
            "wvm": _pack_wv(W_val[hs]).astype(np.float16),
            "wq": np.concatenate(
                [_pack_w(W_query[hs[2 * p:2 * p + 2]] * norm) for p in range(2)],
                axis=1).astype(np.float16),
            "wk": np.concatenate(
                [_pack_w(W_key[hs[2 * p:2 * p + 2]]) for p in range(2)],
                axis=1).astype(np.float16),
            "wo": np.concatenate(
                [_pack_wo(W_out[hs[2 * p:2 * p + 2]]) for p in range(2)],
                axis=1).astype(np.float16),
        }
        in_maps.append(m)

    res = run_bass_kernel_spmd(nc, in_maps, list(range(N_CORES)),
                               trace=_trace)
    parts = [res.results[c]["pout"].astype(np.float32) for c in range(N_CORES)]
    out = np.stack([
        parts[0] + parts[1] + parts[2] + parts[3],
        parts[4] + parts[5] + parts[6] + parts[7],
    ]).astype(np.float32)
    if _trace:
        _CACHE["last_result"] = res
    return out


# revision 36
# speedup vs baseline: 1.4138x; 1.0142x over previous
"""Trainium2 Bass kernel for 16-head MHA (B=2, S=2048, D=1024, E=64).

Sharding: 8 cores = 2 batches x 4 head-groups. Each core computes 4 heads
(2 pairs of 2) for one batch and returns a partial output [2048, 1024]
(sum of its 4 heads' contributions after the output projection). Host sums
the 4 partials per batch.

Per-core pipeline (all matmuls on PE, fp32 PSUM accumulation):
  - K/Q projections feature-major (weights stationary, x moving)
  - V projection token-major directly on the PE (x chunk stationary,
    W_val moving) -- avoids DMA transposes entirely
  - S^T = K Q^T per head pair, two heads row-packed in the 128x128 array
  - A^T = exp(S^T) on ACT (scale folded into W_query on host); ACT does
    ONLY exp -- all psum evacuations go through DVE
  - O^T accumulation with fused row-sum via a ones column in the V tiles
  - softmax normalization: DVE reciprocal_approx_fast + GPSIMD
    partition-broadcast + DVE multiply (writes fp16 O^T)
  - output projection (fp16) accumulating both pairs, fp16 partials out
  - phase 1 is software-pipelined into attention: slice-ordered DMAs,
    K proj first, V-blocks + Q-slices interleaved with attention qc=0
"""

import sys

sys.path.insert(0, "/opt/trn_rl_repo")

import numpy as np

import concourse.bass as bass
import concourse.bacc as bacc
import concourse.mybir as mybir
from concourse import tile
from concourse.tile_rust import add_dep_helper
from concourse.bass_interp import get_hw_module
from concourse.bass_utils import run_bass_kernel_spmd

F16 = mybir.dt.float16
F32 = mybir.dt.float32
BF16 = mybir.dt.bfloat16
I16 = mybir.dt.int16

# Schraudolph exp: bf16 bits = round(x * 128/ln2 + B); B tuned for zero mean
# relative error so softmax numerator/denominator biases cancel
SCHRAUD_A = float(np.float32(128.0 / np.log(2.0)))
SCHRAUD_B = 16250.0

N_CORES = 8
T = 2048          # tokens per core (one batch)
D = 1024          # model dim
E = 64            # head dim
QC = 512          # query chunk
NQ = T // QC      # 4 query chunks
KB = 128          # key block
NKB = T // KB     # 16 key blocks
ND = D // 128     # 8 contraction chunks for projections
NS = 4            # token slices (512 each)

_CACHE = {}


def _build():
    nc = bacc.Bacc("TRN2", target_bir_lowering=False, debug=False,
                   num_devices=N_CORES)

    xqT = nc.dram_tensor("xqT", [D, T], F16, kind="ExternalInput").ap()
    xkT = nc.dram_tensor("xkT", [D, T], F16, kind="ExternalInput").ap()
    xvT = nc.dram_tensor("xvT", [D, T], F16, kind="ExternalInput").ap()
    # packed weights, both pairs side by side: pair p at cols p*D, within a
    # pair chunk d at cols d*128
    wq = nc.dram_tensor("wq", [128, 2 * D], F16, kind="ExternalInput").ap()
    wk = nc.dram_tensor("wk", [128, 2 * D], F16, kind="ExternalInput").ap()
    # V weights as moving operand: [128, 8*256]; chunk d at cols d*256,
    # within a chunk cols h*64:(h+1)*64 = head h features
    wvm = nc.dram_tensor("wvm", [128, 8 * 256], F16, kind="ExternalInput").ap()
    wo = nc.dram_tensor("wo", [128, 2 * D], F16, kind="ExternalInput").ap()
    pout = nc.dram_tensor("pout", [T, D], F16, kind="ExternalOutput").ap()

    with tile.TileContext(nc) as tc:
        with (
            tc.tile_pool(name="consts", bufs=1) as consts,
            tc.tile_pool(name="persist", bufs=1) as persist,
            tc.tile_pool(name="xs", bufs=1) as xs,
            tc.tile_pool(name="at", bufs=8) as atp,
            tc.tile_pool(name="o2t", bufs=2) as o2tp,
            tc.tile_pool(name="os", bufs=3) as osp,
            tc.tile_pool(name="small", bufs=2) as smallp,
            tc.tile_pool(name="psS", bufs=2, space="PSUM") as psS,
            tc.tile_pool(name="psO", bufs=1, space="PSUM") as psO,
            tc.tile_pool(name="psP", bufs=2, space="PSUM") as psP,
        ):
            # ---- weights (descriptor gen spread across idle engine queues) ----
            wq_sb = consts.tile([128, 2 * D], F16, tag="wq", name="wq_sb")
            wk_sb = consts.tile([128, 2 * D], F16, tag="wk", name="wk_sb")
            wv_sb = consts.tile([128, 8 * 256], F16, tag="wvm", name="wv_sb")
            wo_sb = consts.tile([128, 2 * D], F16, tag="wo", name="wo_sb")
            nc.sync.dma_start(wq_sb[:], wq[:])
            nc.gpsimd.dma_start(wk_sb[:], wk[:])
            nc.scalar.dma_start(wv_sb[:], wvm[:])
            nc.scalar.dma_start(wo_sb[:], wo[:])

            # ---- persistent activations ----
            # feature-major Q^T, K^T per pair: rows 0:64 head-even, 64:128 head-odd
            qt = [[persist.tile([128, QC], F16, tag=f"qt{p}_{t}", name=f"qt{p}_{t}")
                   for t in range(NQ)] for p in range(2)]
            kt = [persist.tile([128, T], F16, tag=f"kt{p}", name=f"kt{p}") for p in range(2)]
            # token-major [V_even | 1 | V_odd | 1] per (pair, key-block): [128, 130]
            v2 = [[persist.tile([128, 130], BF16, tag=f"v2_{p}_{b}", name=f"v2_{p}_{b}")
                   for b in range(NKB)] for p in range(2)]
            for p in range(2):
                for b in range(NKB):
                    nc.vector.memset(
                        v2[p][b].rearrange("p (c n) -> p c n", c=2)[:, :, 64:65], 1.0)

            # ---- input DMAs: full [128, T] chunks, ordered k, q, v; two
            # engine queues alternate so descriptor gen is not serial
            def load_chunks(x_dram, pfx):
                ts = []
                for d in range(ND):
                    t = xs.tile([128, T], F16, tag=f"{pfx}{d}", name=f"{pfx}{d}")
                    eng = nc.sync if d % 2 == 0 else nc.gpsimd
                    eng.dma_start(t[:], x_dram[d * 128:(d + 1) * 128, :])
                    ts.append(t)
                return ts

            xk_t = load_chunks(xkT, "xk")
            xq_t = load_chunks(xqT, "xq")
            xv_t = load_chunks(xvT, "xv")

            # ---- HAM warmup: junk matmuls on a memset tile (no DMA dep) fill
            # the input-DMA hole and flip the PE clock gate to 8/8 before the
            # real projections start
            junk = consts.tile([128, QC], F16, tag="junk", name="junk")
            nc.vector.memset(junk[:], 0.5)
            warm = psP.tile([128, QC], F32, tag="pp", name="warm")
            for i in range(16):
                nc.tensor.matmul(warm[:], junk[:, 0:128],
                                 junk[:], start=True, stop=True)

            # ---- projection helpers ----
            def proj_fm_slice(x_tiles, w_sb, s, evac):
                # feature-major: weights stationary, x moving; psum per pair
                for p in range(2):
                    ps = psP.tile([128, QC], F32, tag="pp", name=f"pj_{id(x_tiles)}_{s}_{p}")
                    for d in range(ND):
                        nc.tensor.matmul(
                            ps[:], w_sb[:, p * D + d * 128:p * D + (d + 1) * 128],
                            x_tiles[d][:, s * QC:(s + 1) * QC],
                            start=(d == 0), stop=(d == ND - 1))
                    evac(p, s, ps)

            def evac_kt(p, s, ps):
                nc.vector.tensor_copy(kt[p][:, s * QC:(s + 1) * QC], ps[:])

            def evac_qt(p, s, ps):
                nc.vector.tensor_copy(qt[p][s][:], ps[:])

            def proj_v_block(b):
                # token-major V: x chunk slice stationary, W_val moving
                ps = psP.tile([128, 256], F32, tag="pp", name=f"pv_{b}")
                for d in range(ND):
                    nc.tensor.matmul(
                        ps[:],
                        xv_t[d][:, b * 128:(b + 1) * 128],
                        wv_sb[:, d * 256:(d + 1) * 256],
                        start=(d == 0), stop=(d == ND - 1))
                for p in range(2):
                    nc.vector.tensor_copy(
                        v2[p][b].rearrange("p (c n) -> p c n", c=2)[:, :, 0:64],
                        ps[:, p * 128:(p + 1) * 128].rearrange(
                            "p (c n) -> p c n", c=2))

            # ---- attention + output projection ----
            ost_live = {}

            def emit_outproj_group(qc, o2t, sub, oc, anchor):
                q0 = qc * QC
                if oc == 0:
                    ost_live[(qc, sub)] = osp.tile(
                        [128, D], F16, tag="os", name=f"os_{qc}_{sub}")
                ost = ost_live[(qc, sub)]
                pp = psP.tile([128, 512], F32, tag="pp", name=f"pp_{qc}_{sub}_{oc}")
                for p in range(2):
                    mm = nc.tensor.matmul(
                        pp[:],
                        o2t[p][:, sub * 128:(sub + 1) * 128],
                        wo_sb[:, p * D + oc * 512:p * D + (oc + 1) * 512],
                        start=(p == 0), stop=(p == 1))
                    if p == 0 and anchor is not None:
                        add_dep_helper(mm.ins, anchor.ins, sync=False,
                                       reason="interleave outproj after S")
                nc.vector.tensor_copy(
                    ost[:, oc * 512:(oc + 1) * 512], pp[:])
                if oc == 1:
                    nc.sync.dma_start(
                        pout[q0 + sub * 128:q0 + (sub + 1) * 128, :],
                        ost[:])
                    del ost_live[(qc, sub)]

            def emit_outproj(qc, o2t, anchor=None):
                for sub in range(4):
                    for oc in range(2):
                        emit_outproj_group(qc, o2t, sub, oc, anchor)

            po_live = {}

            def attn_s_exp(qc, p, kb):
                # S^T matmul pair then exp for one key block
                k0 = kb * KB
                ps = psS.tile([128, 2 * QC], F32, tag="s", name=f"s_{qc}_{p}_{kb}")
                s_anchor = nc.tensor.matmul(
                    ps[:, 0:QC],
                    kt[p][0:64, k0:k0 + KB],
                    qt[p][qc][0:64, :],
                    start=True, stop=True, tile_position=(0, 0))
                nc.tensor.matmul(
                    ps[:, QC:2 * QC],
                    kt[p][64:128, k0:k0 + KB],
                    qt[p][qc][64:128, :],
                    start=True, stop=True, tile_position=(64, 0))
                at = atp.tile([128, 2 * QC], BF16, tag="at", name=f"at_{qc}_{p}_{kb}")
                nc.scalar.activation(
                    at[:], ps[:], mybir.ActivationFunctionType.Exp)
                return at, s_anchor

            def attn_v(qc, p, kb, at):
                po = po_live[(qc, p)]
                for h in range(2):
                    nc.tensor.matmul(
                        po[h][:],
                        v2[p][kb][:, h * 65:h * 65 + 65],
                        at[:, h * QC:(h + 1) * QC],
                        start=(kb == 0), stop=(kb == NKB - 1))

            def attn_kb(qc, p, kb, pending):
                at, s_anchor = attn_s_exp(qc, p, kb)
                attn_v(qc, p, kb, at)
                if p == 1 and pending is not None and kb % 2 == 1:
                    pqc, po2t = pending
                    emit_outproj_group(pqc, po2t, kb // 4, (kb // 2) % 2,
                                       s_anchor)

            def attn_norm(qc, p, o2t):
                # softmax normalization for both heads of pair p
                # (po rows 0:64 = O^T, row 64 = denominator)
                po = po_live.pop((qc, p))
                ots, rbs = [], []
                for h in range(2):
                    # denominator chain first -- it has the longest latency
                    # (recip -> gpsimd broadcast) before the final multiply
                    den = smallp.tile([1, QC], F32, tag=f"den{h}", name=f"den_{qc}_{p}_{h}")
                    nc.vector.tensor_copy(den[:], po[h][64:65, :])
                    r = smallp.tile([1, QC], F32, tag=f"r{h}", name=f"r_{qc}_{p}_{h}")
                    nc.vector.reciprocal_approx_fast(r[:], den[:])
                    rb = smallp.tile([64, QC], F32, tag=f"rb{h}", name=f"rb_{qc}_{p}_{h}")
                    nc.gpsimd.partition_broadcast(rb[:], r[:])
                    rbs.append(rb)
                for h in range(2):
                    ot = smallp.tile([64, QC], F32, tag=f"ot{h}", name=f"ot_{qc}_{p}_{h}")
                    nc.vector.tensor_copy(ot[:], po[h][0:64, :])
                    ots.append(ot)
                for h in range(2):
                    nc.vector.tensor_mul(
                        o2t[h * 64:(h + 1) * 64, :],
                        ots[h][:], rbs[h][:])

            def new_po(qc, p):
                po_live[(qc, p)] = [
                    psO.tile([65, QC], F32, tag=f"o{h}", name=f"po_{qc}_{p}_{h}")
                    for h in range(2)]

            # ---- emission schedule ----
            # K projection first (all slices)
            for s in range(NS):
                proj_fm_slice(xk_t, wk_sb, s, evac_kt)

            # qc=0 attention (pair 0) interleaved with V blocks and Q slices
            o2t_cur = [o2tp.tile([128, QC], F16, tag=f"o2t{p}", name=f"o2t_0_{p}")
                       for p in range(2)]
            new_po(0, 0)
            for g in range(4):
                if g == 0:
                    proj_fm_slice(xq_t, wq_sb, 0, evac_qt)
                # S+exp first (no xv dependency -- keeps ACT streaming even
                # while xv data is still arriving), then V blocks, then attnV
                ats = [attn_s_exp(0, 0, kb)[0] for kb in range(4 * g, 4 * g + 4)]
                for b in range(4 * g, 4 * g + 4):
                    proj_v_block(b)
                for kb, at in zip(range(4 * g, 4 * g + 4), ats):
                    attn_v(0, 0, kb, at)
                if g > 0:
                    proj_fm_slice(xq_t, wq_sb, g, evac_qt)
            attn_norm(0, 0, o2t_cur[0])
            new_po(0, 1)
            for kb in range(NKB):
                attn_kb(0, 1, kb, None)
            attn_norm(0, 1, o2t_cur[1])
            pending = (0, o2t_cur)

            # qc=1..3 with previous qc's outproj interleaved into pair 1
            for qc in range(1, NQ):
                o2t_cur = [o2tp.tile([128, QC], F16, tag=f"o2t{p}", name=f"o2t_{qc}_{p}")
                           for p in range(2)]
                for p in range(2):
                    new_po(qc, p)
                    for kb in range(NKB):
                        attn_kb(qc, p, kb, pending if p == 1 else None)
                    attn_norm(qc, p, o2t_cur[p])
                pending = (qc, o2t_cur)
            emit_outproj(*pending)

    nc.compile()
    nc.m = get_hw_module(nc.m)
    return nc


def _pack_w(w_pair):
    # w_pair: [2, 1024, 64] -> [1024, 128] -> chunk-major [128, 8*128]
    w = np.concatenate([w_pair[0], w_pair[1]], axis=1)          # [1024, 128]
    return np.ascontiguousarray(
        w.reshape(ND, 128, 128).transpose(1, 0, 2).reshape(128, D))


def _pack_wv(w4):
    # w4: [4, 1024, 64] -> [1024, 256] -> chunk-major [128, 8*256]
    w = np.concatenate([w4[h] for h in range(4)], axis=1)       # [1024, 256]
    return np.ascontiguousarray(
        w.reshape(ND, 128, 256).transpose(1, 0, 2).reshape(128, ND * 256))


def _pack_wo(wo_pair):
    # wo_pair: [2, 64, 1024] -> [128, 1024]
    return np.ascontiguousarray(np.concatenate([wo_pair[0], wo_pair[1]], axis=0))


def kernel(q, k, v, W_query, W_key, W_val, W_out, _trace=False):
    q = np.asarray(q, dtype=np.float32)
    k = np.asarray(k, dtype=np.float32)
    v = np.asarray(v, dtype=np.float32)
    W_query = np.asarray(W_query, dtype=np.float32)
    W_key = np.asarray(W_key, dtype=np.float32)
    W_val = np.asarray(W_val, dtype=np.float32)
    W_out = np.asarray(W_out, dtype=np.float32)

    if "nc" not in _CACHE:
        _CACHE["nc"] = _build()
    nc = _CACHE["nc"]

    norm = 1.0 / np.sqrt(E)
    xT = {}
    for b in range(2):
        xT[("q", b)] = np.ascontiguousarray(q[b].T).astype(np.float16)
        xT[("k", b)] = np.ascontiguousarray(k[b].T).astype(np.float16)
        xT[("v", b)] = np.ascontiguousarray(v[b].T).astype(np.float16)

    in_maps = []
    for c in range(N_CORES):
        b, g = c // 4, c % 4
        hs = [4 * g, 4 * g + 1, 4 * g + 2, 4 * g + 3]
        m = {
            "xqT": xT[("q", b)], "xkT": xT[("k", b)], "xvT": xT[("v", b)],
            "wvm": _pack_wv(W_val[hs]).astype(np.float16),
            "wq": np.concatenate(
                [_pack_w(W_query[hs[2 * p:2 * p + 2]] * norm) for p in range(2)],
                axis=1).astype(np.float16),
            "wk": np.concatenate(
                [_pack_w(W_key[hs[2 * p:2 * p + 2]]) for p in range(2)],
                axis=1).astype(np.float16),
            "wo": np.concatenate(
                [_pack_wo(W_out[hs[2 * p:2 * p + 2]]) for p in range(2)],
                axis=1).astype(np.float16),
        }
        in_maps.append(m)

    res = run_bass_kernel_spmd(nc, in_maps, list(range(N_CORES)),
                               trace=_trace)
    parts = [res.results[c]["pout"].astype(np.float32) for c in range(N_CORES)]
    out = np.stack([
        parts[0] + parts[1] + parts[2] + parts[3],
        parts[4] + parts[5] + parts[6] + parts[7],
    ]).astype(np.float32)
    if _trace:
        _CACHE["last_result"] = res
    return out


# revision 42
# speedup vs baseline: 1.5035x; 1.0634x over previous
"""Trainium2 Bass kernel for 16-head MHA (B=2, S=2048, D=1024, E=64).

Sharding: 8 cores = 2 batches x 4 head-groups. Each core computes 4 heads
(2 pairs of 2) for one batch and returns a partial output [2048, 1024]
(sum of its 4 heads' contributions after the output projection). Host sums
the 4 partials per batch.

Per-core pipeline (all matmuls on PE, fp32 PSUM accumulation):
  - K/Q projections feature-major (weights stationary, x moving)
  - V projection token-major directly on the PE (x chunk stationary,
    W_val moving) -- avoids DMA transposes entirely
  - S^T = K Q^T per head pair, two heads row-packed in the 128x128 array
  - A^T = exp(S^T) on ACT (scale folded into W_query on host); ACT does
    ONLY exp -- all psum evacuations go through DVE
  - O^T accumulation with fused row-sum via a ones column in the V tiles
  - softmax normalization: DVE reciprocal_approx_fast + GPSIMD
    partition-broadcast + DVE multiply (writes fp16 O^T)
  - output projection (fp16) accumulating both pairs, fp16 partials out
  - phase 1 is software-pipelined into attention: slice-ordered DMAs,
    K proj first, V-blocks + Q-slices interleaved with attention qc=0
"""

import sys

sys.path.insert(0, "/opt/trn_rl_repo")

import numpy as np

import concourse.bass as bass
import concourse.bacc as bacc
import concourse.mybir as mybir
from concourse import tile
from concourse.tile_rust import add_dep_helper
from concourse.bass_interp import get_hw_module
from concourse.bass_utils import run_bass_kernel_spmd

F16 = mybir.dt.float16
F32 = mybir.dt.float32
BF16 = mybir.dt.bfloat16
I16 = mybir.dt.int16

# Schraudolph exp: bf16 bits = round(x * 128/ln2 + B); B tuned for zero mean
# relative error so softmax numerator/denominator biases cancel
SCHRAUD_A = float(np.float32(128.0 / np.log(2.0)))
SCHRAUD_B = 16250.0

N_CORES = 8
T = 2048          # tokens per core (one batch)
D = 1024          # model dim
E = 64            # head dim
QC = 512          # query chunk
NQ = T // QC      # 4 query chunks
KB = 128          # key block
NKB = T // KB     # 16 key blocks
ND = D // 128     # 8 contraction chunks for projections
NS = 4            # token slices (512 each)

_CACHE = {}


def _build():
    nc = bacc.Bacc("TRN2", target_bir_lowering=False, debug=False,
                   num_devices=N_CORES)

    xqT = nc.dram_tensor("xqT", [D, T], F16, kind="ExternalInput").ap()
    xkT = nc.dram_tensor("xkT", [D, T], F16, kind="ExternalInput").ap()
    xvT = nc.dram_tensor("xvT", [D, T], F16, kind="ExternalInput").ap()
    # packed weights, both pairs side by side: pair p at cols p*D, within a
    # pair chunk d at cols d*128
    wq = nc.dram_tensor("wq", [128, 2 * D], F16, kind="ExternalInput").ap()
    wk = nc.dram_tensor("wk", [128, 2 * D], F16, kind="ExternalInput").ap()
    # V weights as moving operand: [128, 8*256]; chunk d at cols d*256,
    # within a chunk cols h*64:(h+1)*64 = head h features
    wvm = nc.dram_tensor("wvm", [128, 8 * 256], F16, kind="ExternalInput").ap()
    wo = nc.dram_tensor("wo", [128, 2 * D], F16, kind="ExternalInput").ap()
    pout = nc.dram_tensor("pout", [T, D], F16, kind="ExternalOutput").ap()

    with tile.TileContext(nc) as tc:
        with (
            tc.tile_pool(name="consts", bufs=1) as consts,
            tc.tile_pool(name="persist", bufs=1) as persist,
            tc.tile_pool(name="xs", bufs=1) as xs,
            tc.tile_pool(name="at", bufs=8) as atp,
            tc.tile_pool(name="o2t", bufs=2) as o2tp,
            tc.tile_pool(name="os", bufs=3) as osp,
            tc.tile_pool(name="small", bufs=2) as smallp,
            tc.tile_pool(name="psS", bufs=2, space="PSUM") as psS,
            tc.tile_pool(name="psO", bufs=1, space="PSUM") as psO,
            tc.tile_pool(name="psP", bufs=2, space="PSUM") as psP,
        ):
            # ---- weights (descriptor gen spread across idle engine queues) ----
            wq_sb = consts.tile([128, 2 * D], F16, tag="wq", name="wq_sb")
            wk_sb = consts.tile([128, 2 * D], F16, tag="wk", name="wk_sb")
            wv_sb = consts.tile([128, 8 * 256], F16, tag="wvm", name="wv_sb")
            wo_sb = consts.tile([128, 2 * D], F16, tag="wo", name="wo_sb")
            nc.sync.dma_start(wq_sb[:], wq[:])
            nc.gpsimd.dma_start(wk_sb[:], wk[:])
            nc.scalar.dma_start(wv_sb[:], wvm[:])
            nc.scalar.dma_start(wo_sb[:], wo[:])

            # ---- persistent activations ----
            # feature-major Q^T, K^T per pair: rows 0:64 head-even, 64:128 head-odd
            qt = [[persist.tile([128, QC], F16, tag=f"qt{p}_{t}", name=f"qt{p}_{t}")
                   for t in range(NQ)] for p in range(2)]
            kt = [persist.tile([128, T], F16, tag=f"kt{p}", name=f"kt{p}") for p in range(2)]
            # token-major [V_even | 1 | V_odd | 1] per (pair, key-block): [128, 130]
            v2 = [[persist.tile([128, 130], BF16, tag=f"v2_{p}_{b}", name=f"v2_{p}_{b}")
                   for b in range(NKB)] for p in range(2)]
            for p in range(2):
                for b in range(NKB):
                    nc.vector.memset(
                        v2[p][b].rearrange("p (c n) -> p c n", c=2)[:, :, 64:65], 1.0)

            # ---- input DMAs: slice-granular [128, 512] tiles, interleaved
            # k,q,v per token-slice so the first attention slice only waits
            # for 2.5MB instead of 9.5MB; two engine queues alternate so
            # descriptor generation keeps ahead of the data rate
            def alloc_slices(pfx):
                return [[xs.tile([128, QC], F16, tag=f"{pfx}{d}_{s}",
                                 name=f"{pfx}{d}_{s}") for s in range(NS)]
                        for d in range(ND)]

            xk_t = alloc_slices("xk")
            xq_t = alloc_slices("xq")
            xv_t = alloc_slices("xv")

            def load_slice(ts, x_dram, s):
                for d in range(ND):
                    eng = nc.sync if d % 2 == 0 else nc.gpsimd
                    eng.dma_start(ts[d][s][:],
                                  x_dram[d * 128:(d + 1) * 128,
                                         s * QC:(s + 1) * QC])

            for s in range(NS):
                load_slice(xk_t, xkT, s)
                load_slice(xq_t, xqT, s)
                load_slice(xv_t, xvT, s)

            # ---- HAM warmup: junk matmuls on a memset tile (no DMA dep) fill
            # the input-DMA hole and flip the PE clock gate to 8/8 before the
            # real projections start
            junk = consts.tile([128, QC], F16, tag="junk", name="junk")
            nc.vector.memset(junk[:], 0.5)
            warm = psP.tile([128, QC], F32, tag="pp", name="warm")
            for i in range(16):
                nc.tensor.matmul(warm[:], junk[:, 0:128],
                                 junk[:], start=True, stop=True)

            # ---- projection helpers ----
            def proj_fm_slice(x_tiles, w_sb, s, evac):
                # feature-major: weights stationary, x moving; psum per pair
                for p in range(2):
                    ps = psP.tile([128, QC], F32, tag="pp", name=f"pj_{id(x_tiles)}_{s}_{p}")
                    for d in range(ND):
                        nc.tensor.matmul(
                            ps[:], w_sb[:, p * D + d * 128:p * D + (d + 1) * 128],
                            x_tiles[d][s][:],
                            start=(d == 0), stop=(d == ND - 1))
                    evac(p, s, ps)

            def evac_kt(p, s, ps):
                nc.vector.tensor_copy(kt[p][:, s * QC:(s + 1) * QC], ps[:])

            def evac_qt(p, s, ps):
                nc.vector.tensor_copy(qt[p][s][:], ps[:])

            def proj_v_block(b):
                # token-major V: x chunk slice stationary, W_val moving
                ps = psP.tile([128, 256], F32, tag="pp", name=f"pv_{b}")
                s, j = b // 4, b % 4
                for d in range(ND):
                    nc.tensor.matmul(
                        ps[:],
                        xv_t[d][s][:, j * 128:(j + 1) * 128],
                        wv_sb[:, d * 256:(d + 1) * 256],
                        start=(d == 0), stop=(d == ND - 1))
                for p in range(2):
                    nc.vector.tensor_copy(
                        v2[p][b].rearrange("p (c n) -> p c n", c=2)[:, :, 0:64],
                        ps[:, p * 128:(p + 1) * 128].rearrange(
                            "p (c n) -> p c n", c=2))

            # ---- attention + output projection ----
            ost_live = {}

            def emit_outproj_group(qc, o2t, sub, oc, anchor):
                q0 = qc * QC
                if oc == 0:
                    ost_live[(qc, sub)] = osp.tile(
                        [128, D], F16, tag="os", name=f"os_{qc}_{sub}")
                ost = ost_live[(qc, sub)]
                pp = psP.tile([128, 512], F32, tag="pp", name=f"pp_{qc}_{sub}_{oc}")
                for p in range(2):
                    mm = nc.tensor.matmul(
                        pp[:],
                        o2t[p][:, sub * 128:(sub + 1) * 128],
                        wo_sb[:, p * D + oc * 512:p * D + (oc + 1) * 512],
                        start=(p == 0), stop=(p == 1))
                    if p == 0 and anchor is not None:
                        add_dep_helper(mm.ins, anchor.ins, sync=False,
                                       reason="interleave outproj after S")
                nc.vector.tensor_copy(
                    ost[:, oc * 512:(oc + 1) * 512], pp[:])
                if oc == 1:
                    nc.sync.dma_start(
                        pout[q0 + sub * 128:q0 + (sub + 1) * 128, :],
                        ost[:])
                    del ost_live[(qc, sub)]

            def emit_outproj(qc, o2t, anchor=None):
                for sub in range(4):
                    for oc in range(2):
                        emit_outproj_group(qc, o2t, sub, oc, anchor)

            po_live = {}

            def attn_s_exp(qc, p, kb):
                # S^T matmul pair then exp for one key block
                k0 = kb * KB
                ps = psS.tile([128, 2 * QC], F32, tag="s", name=f"s_{qc}_{p}_{kb}")
                s_anchor = nc.tensor.matmul(
                    ps[:, 0:QC],
                    kt[p][0:64, k0:k0 + KB],
                    qt[p][qc][0:64, :],
                    start=True, stop=True, tile_position=(0, 0))
                nc.tensor.matmul(
                    ps[:, QC:2 * QC],
                    kt[p][64:128, k0:k0 + KB],
                    qt[p][qc][64:128, :],
                    start=True, stop=True, tile_position=(64, 0))
                at = atp.tile([128, 2 * QC], BF16, tag="at", name=f"at_{qc}_{p}_{kb}")
                nc.scalar.activation(
                    at[:], ps[:], mybir.ActivationFunctionType.Exp)
                return at, s_anchor

            def attn_v(qc, p, kb, at):
                po = po_live[(qc, p)]
                for h in range(2):
                    nc.tensor.matmul(
                        po[h][:],
                        v2[p][kb][:, h * 65:h * 65 + 65],
                        at[:, h * QC:(h + 1) * QC],
                        start=(kb == 0), stop=(kb == NKB - 1))

            def attn_kb(qc, p, kb, pending):
                at, s_anchor = attn_s_exp(qc, p, kb)
                attn_v(qc, p, kb, at)
                # interleave 7 of the previous qc's 8 outproj groups from kb=3
                # on (anchor later than kb=1 so the normalization chain of the
                # previous pair has slack); the 8th is emitted after the loop
                if p == 1 and pending is not None and kb >= 3 and kb % 2 == 1:
                    idx = (kb - 3) // 2
                    pqc, po2t = pending
                    emit_outproj_group(pqc, po2t, idx // 2, idx % 2, s_anchor)

            def attn_norm(qc, p, o2t):
                # softmax normalization for both heads of pair p
                # (po rows 0:64 = O^T, row 64 = denominator)
                po = po_live.pop((qc, p))
                ots, rbs = [], []
                for h in range(2):
                    # denominator chain first -- it has the longest latency
                    # (recip -> gpsimd broadcast) before the final multiply
                    den = smallp.tile([1, QC], F32, tag=f"den{h}", name=f"den_{qc}_{p}_{h}")
                    nc.vector.tensor_copy(den[:], po[h][64:65, :])
                    r = smallp.tile([1, QC], F32, tag=f"r{h}", name=f"r_{qc}_{p}_{h}")
                    nc.vector.reciprocal_approx_fast(r[:], den[:])
                    rb = smallp.tile([64, QC], F32, tag=f"rb{h}", name=f"rb_{qc}_{p}_{h}")
                    nc.gpsimd.partition_broadcast(rb[:], r[:])
                    rbs.append(rb)
                for h in range(2):
                    ot = smallp.tile([64, QC], F32, tag=f"ot{h}", name=f"ot_{qc}_{p}_{h}")
                    nc.vector.tensor_copy(ot[:], po[h][0:64, :])
                    ots.append(ot)
                for h in range(2):
                    nc.vector.tensor_mul(
                        o2t[h * 64:(h + 1) * 64, :],
                        ots[h][:], rbs[h][:])

            def new_po(qc, p):
                po_live[(qc, p)] = [
                    psO.tile([65, QC], F32, tag=f"o{h}", name=f"po_{qc}_{p}_{h}")
                    for h in range(2)]

            # ---- emission schedule ----
            # prologue: first K and Q slices, then qc0-p0 attention groups
            # interleaved with remaining K slices and V blocks; Q s1-3 are
            # emitted later as PE filler during steady-state attention
            proj_fm_slice(xk_t, wk_sb, 0, evac_kt)
            proj_fm_slice(xq_t, wq_sb, 0, evac_qt)

            o2t_cur = [o2tp.tile([128, QC], F16, tag=f"o2t{p}", name=f"o2t_0_{p}")
                       for p in range(2)]
            new_po(0, 0)
            for g in range(4):
                # S+exp first (no xv dependency -- keeps ACT streaming even
                # while xv data is still arriving), then V blocks, then attnV
                ats = [attn_s_exp(0, 0, kb)[0] for kb in range(4 * g, 4 * g + 4)]
                for b in range(4 * g, 4 * g + 4):
                    proj_v_block(b)
                for kb, at in zip(range(4 * g, 4 * g + 4), ats):
                    attn_v(0, 0, kb, at)
                if g < 3:
                    proj_fm_slice(xk_t, wk_sb, g + 1, evac_kt)
                else:
                    proj_fm_slice(xq_t, wq_sb, 1, evac_qt)
            attn_norm(0, 0, o2t_cur[0])
            new_po(0, 1)
            for kb in range(NKB):
                attn_kb(0, 1, kb, None)
            attn_norm(0, 1, o2t_cur[1])
            pending = (0, o2t_cur)

            # qc=1..3 with previous qc's outproj interleaved into pair 1
            # (7 groups anchored in the kb loop, the 8th right after)
            for qc in range(1, NQ):
                o2t_cur = [o2tp.tile([128, QC], F16, tag=f"o2t{p}", name=f"o2t_{qc}_{p}")
                           for p in range(2)]
                for p in range(2):
                    new_po(qc, p)
                    for kb in range(NKB):
                        attn_kb(qc, p, kb, pending if p == 1 else None)
                    if p == 1:
                        emit_outproj_group(pending[0], pending[1], 3, 1, None)
                    attn_norm(qc, p, o2t_cur[p])
                    if qc == 1:
                        proj_fm_slice(xq_t, wq_sb, 2 + p, evac_qt)
                pending = (qc, o2t_cur)
            emit_outproj(*pending)

    nc.compile()
    nc.m = get_hw_module(nc.m)
    return nc


def _pack_w(w_pair):
    # w_pair: [2, 1024, 64] -> [1024, 128] -> chunk-major [128, 8*128]
    w = np.concatenate([w_pair[0], w_pair[1]], axis=1)          # [1024, 128]
    return np.ascontiguousarray(
        w.reshape(ND, 128, 128).transpose(1, 0, 2).reshape(128, D))


def _pack_wv(w4):
    # w4: [4, 1024, 64] -> [1024, 256] -> chunk-major [128, 8*256]
    w = np.concatenate([w4[h] for h in range(4)], axis=1)       # [1024, 256]
    return np.ascontiguousarray(
        w.reshape(ND, 128, 256).transpose(1, 0, 2).reshape(128, ND * 256))


def _pack_wo(wo_pair):
    # wo_pair: [2, 64, 1024] -> [128, 1024]
    return np.ascontiguousarray(np.concatenate([wo_pair[0], wo_pair[1]], axis=0))


def kernel(q, k, v, W_query, W_key, W_val, W_out, _trace=False):
    q = np.asarray(q, dtype=np.float32)
    k = np.asarray(k, dtype=np.float32)
    v = np.asarray(v, dtype=np.float32)
    W_query = np.asarray(W_query, dtype=np.float32)
    W_key = np.asarray(W_key, dtype=np.float32)
    W_val = np.asarray(W_val, dtype=np.float32)
    W_out = np.asarray(W_out, dtype=np.float32)

    if "nc" not in _CACHE:
        _CACHE["nc"] = _build()
    nc = _CACHE["nc"]

    norm = 1.0 / np.sqrt(E)
    xT = {}
    for b in range(2):
        xT[("q", b)] = np.ascontiguousarray(q[b].T).astype(np.float16)
        xT[("k", b)] = np.ascontiguousarray(k[b].T).astype(np.float16)
        xT[("v", b)] = np.ascontiguousarray(v[b].T).astype(np.float16)

    in_maps = []
    for c in range(N_CORES):
        b, g = c // 4, c % 4
        hs = [4 * g, 4 * g + 1, 4 * g + 2, 4 * g + 3]
        m = {
            "xqT": xT[("q", b)], "xkT": xT[("k", b)], "xvT": xT[("v", b)],
            "wvm": _pack_wv(W_val[hs]).astype(np.float16),
            "wq": np.concatenate(
                [_pack_w(W_query[hs[2 * p:2 * p + 2]] * norm) for p in range(2)],
                axis=1).astype(np.float16),
            "wk": np.concatenate(
                [_pack_w(W_key[hs[2 * p:2 * p + 2]]) for p in range(2)],
                axis=1).astype(np.float16),
            "wo": np.concatenate(
                [_pack_wo(W_out[hs[2 * p:2 * p + 2]]) for p in range(2)],
                axis=1).astype(np.float16),
        }
        in_maps.append(m)

    res = run_bass_kernel_spmd(nc, in_maps, list(range(N_CORES)),
                               trace=_trace)
    parts = [res.results[c]["pout"].astype(np.float32) for c in range(N_CORES)]
    out = np.stack([
        parts[0] + parts[1] + parts[2] + parts[3],
        parts[4] + parts[5] + parts[6] + parts[7],
    ]).astype(np.float32)
    if _trace:
        _CACHE["last_result"] = res
    return out


# revision 43
# speedup vs baseline: 1.5275x; 1.0160x over previous
"""Trainium2 Bass kernel for 16-head MHA (B=2, S=2048, D=1024, E=64).

Sharding: 8 cores = 2 batches x 4 head-groups. Each core computes 4 heads
(2 pairs of 2) for one batch and returns a partial output [2048, 1024]
(sum of its 4 heads' contributions after the output projection). Host sums
the 4 partials per batch.

Per-core pipeline (all matmuls on PE, fp32 PSUM accumulation):
  - K/Q projections feature-major (weights stationary, x moving)
  - V projection token-major directly on the PE (x chunk stationary,
    W_val moving) -- avoids DMA transposes entirely
  - S^T = K Q^T per head pair, two heads row-packed in the 128x128 array
  - A^T = exp(S^T) on ACT (scale folded into W_query on host); ACT does
    ONLY exp -- all psum evacuations go through DVE
  - O^T accumulation with fused row-sum via a ones column in the V tiles
  - softmax normalization: DVE reciprocal_approx_fast + GPSIMD
    partition-broadcast + DVE multiply (writes fp16 O^T)
  - output projection (fp16) accumulating both pairs, fp16 partials out
  - phase 1 is software-pipelined into attention: slice-ordered DMAs,
    K proj first, V-blocks + Q-slices interleaved with attention qc=0
"""

import sys

sys.path.insert(0, "/opt/trn_rl_repo")

import numpy as np

import concourse.bass as bass
import concourse.bacc as bacc
import concourse.mybir as mybir
from concourse import tile
from concourse.tile_rust import add_dep_helper
from concourse.bass_interp import get_hw_module
from concourse.bass_utils import run_bass_kernel_spmd

F16 = mybir.dt.float16
F32 = mybir.dt.float32
BF16 = mybir.dt.bfloat16
I16 = mybir.dt.int16

# Schraudolph exp: bf16 bits = round(x * 128/ln2 + B); B tuned for zero mean
# relative error so softmax numerator/denominator biases cancel
SCHRAUD_A = float(np.float32(128.0 / np.log(2.0)))
SCHRAUD_B = 16250.0

N_CORES = 8
T = 2048          # tokens per core (one batch)
D = 1024          # model dim
E = 64            # head dim
QC = 512          # query chunk
NQ = T // QC      # 4 query chunks
KB = 128          # key block
NKB = T // KB     # 16 key blocks
ND = D // 128     # 8 contraction chunks for projections
NS = 4            # token slices (512 each)

_CACHE = {}


def _build():
    nc = bacc.Bacc("TRN2", target_bir_lowering=False, debug=False,
                   num_devices=N_CORES)

    xqT = nc.dram_tensor("xqT", [D, T], F16, kind="ExternalInput").ap()
    xkT = nc.dram_tensor("xkT", [D, T], F16, kind="ExternalInput").ap()
    xvT = nc.dram_tensor("xvT", [D, T], F16, kind="ExternalInput").ap()
    # packed weights, both pairs side by side: pair p at cols p*D, within a
    # pair chunk d at cols d*128
    wq = nc.dram_tensor("wq", [128, 2 * D], F16, kind="ExternalInput").ap()
    wk = nc.dram_tensor("wk", [128, 2 * D], F16, kind="ExternalInput").ap()
    # V weights as moving operand: [128, 8*256]; chunk d at cols d*256,
    # within a chunk cols h*64:(h+1)*64 = head h features
    wvm = nc.dram_tensor("wvm", [128, 8 * 256], F16, kind="ExternalInput").ap()
    wo = nc.dram_tensor("wo", [128, 2 * D], F16, kind="ExternalInput").ap()
    pout = nc.dram_tensor("pout", [T, D], F16, kind="ExternalOutput").ap()

    with tile.TileContext(nc) as tc:
        with (
            tc.tile_pool(name="consts", bufs=1) as consts,
            tc.tile_pool(name="persist", bufs=1) as persist,
            tc.tile_pool(name="xs", bufs=1) as xs,
            tc.tile_pool(name="at", bufs=8) as atp,
            tc.tile_pool(name="o2t", bufs=2) as o2tp,
            tc.tile_pool(name="os", bufs=3) as osp,
            tc.tile_pool(name="small", bufs=2) as smallp,
            tc.tile_pool(name="psS", bufs=2, space="PSUM") as psS,
            tc.tile_pool(name="psO", bufs=1, space="PSUM") as psO,
            tc.tile_pool(name="psP", bufs=2, space="PSUM") as psP,
        ):
            # ---- weights (descriptor gen spread across idle engine queues) ----
            wq_sb = consts.tile([128, 2 * D], F16, tag="wq", name="wq_sb")
            wk_sb = consts.tile([128, 2 * D], F16, tag="wk", name="wk_sb")
            wv_sb = consts.tile([128, 8 * 256], F16, tag="wvm", name="wv_sb")
            wo_sb = consts.tile([128, 2 * D], F16, tag="wo", name="wo_sb")
            nc.sync.dma_start(wq_sb[:], wq[:])
            nc.gpsimd.dma_start(wk_sb[:], wk[:])
            nc.scalar.dma_start(wv_sb[:], wvm[:])
            nc.scalar.dma_start(wo_sb[:], wo[:])

            # ---- persistent activations ----
            # feature-major Q^T, K^T per pair: rows 0:64 head-even, 64:128 head-odd
            qt = [[persist.tile([128, QC], F16, tag=f"qt{p}_{t}", name=f"qt{p}_{t}")
                   for t in range(NQ)] for p in range(2)]
            kt = [persist.tile([128, T], F16, tag=f"kt{p}", name=f"kt{p}") for p in range(2)]
            # token-major [V_even | 1 | V_odd | 1] per (pair, key-block): [128, 130]
            v2 = [[persist.tile([128, 130], BF16, tag=f"v2_{p}_{b}", name=f"v2_{p}_{b}")
                   for b in range(NKB)] for p in range(2)]
            for p in range(2):
                for b in range(NKB):
                    nc.vector.memset(
                        v2[p][b].rearrange("p (c n) -> p c n", c=2)[:, :, 64:65], 1.0)

            # ---- input DMAs: slice-granular [128, 512] tiles, interleaved
            # k,q,v per token-slice so the first attention slice only waits
            # for 2.5MB instead of 9.5MB; two engine queues alternate so
            # descriptor generation keeps ahead of the data rate
            def alloc_slices(pfx):
                return [[xs.tile([128, QC], F16, tag=f"{pfx}{d}_{s}",
                                 name=f"{pfx}{d}_{s}") for s in range(NS)]
                        for d in range(ND)]

            xk_t = alloc_slices("xk")
            xq_t = alloc_slices("xq")
            xv_t = alloc_slices("xv")

            def load_slice(ts, x_dram, s):
                for d in range(ND):
                    eng = nc.sync if d % 2 == 0 else nc.gpsimd
                    eng.dma_start(ts[d][s][:],
                                  x_dram[d * 128:(d + 1) * 128,
                                         s * QC:(s + 1) * QC])

            for s in range(NS):
                load_slice(xk_t, xkT, s)
                load_slice(xq_t, xqT, s)
                load_slice(xv_t, xvT, s)

            # ---- HAM warmup: junk matmuls on a memset tile (no DMA dep) fill
            # the input-DMA hole and flip the PE clock gate to 8/8 before the
            # real projections start
            junk = consts.tile([128, QC], F16, tag="junk", name="junk")
            nc.vector.memset(junk[:], 0.5)
            warm = psP.tile([128, QC], F32, tag="pp", name="warm")
            for i in range(16):
                nc.tensor.matmul(warm[:], junk[:, 0:128],
                                 junk[:], start=True, stop=True)

            # ---- projection helpers ----
            def proj_fm_slice(x_tiles, w_sb, s, evac):
                # feature-major: weights stationary, x moving; psum per pair
                for p in range(2):
                    ps = psP.tile([128, QC], F32, tag="pp", name=f"pj_{id(x_tiles)}_{s}_{p}")
                    for d in range(ND):
                        nc.tensor.matmul(
                            ps[:], w_sb[:, p * D + d * 128:p * D + (d + 1) * 128],
                            x_tiles[d][s][:],
                            start=(d == 0), stop=(d == ND - 1))
                    evac(p, s, ps)

            def evac_kt(p, s, ps):
                nc.vector.tensor_copy(kt[p][:, s * QC:(s + 1) * QC], ps[:])

            def evac_qt(p, s, ps):
                nc.vector.tensor_copy(qt[p][s][:], ps[:])

            def proj_v_block(b):
                # token-major V: x chunk slice stationary, W_val moving
                ps = psP.tile([128, 256], F32, tag="pp", name=f"pv_{b}")
                s, j = b // 4, b % 4
                for d in range(ND):
                    nc.tensor.matmul(
                        ps[:],
                        xv_t[d][s][:, j * 128:(j + 1) * 128],
                        wv_sb[:, d * 256:(d + 1) * 256],
                        start=(d == 0), stop=(d == ND - 1))
                for p in range(2):
                    nc.vector.tensor_copy(
                        v2[p][b].rearrange("p (c n) -> p c n", c=2)[:, :, 0:64],
                        ps[:, p * 128:(p + 1) * 128].rearrange(
                            "p (c n) -> p c n", c=2))

            # ---- attention + output projection ----
            ost_live = {}

            def emit_outproj_group(qc, o2t, sub, oc, anchor):
                q0 = qc * QC
                if oc == 0:
                    ost_live[(qc, sub)] = osp.tile(
                        [128, D], F16, tag="os", name=f"os_{qc}_{sub}")
                ost = ost_live[(qc, sub)]
                pp = psP.tile([128, 512], F32, tag="pp", name=f"pp_{qc}_{sub}_{oc}")
                for p in range(2):
                    mm = nc.tensor.matmul(
                        pp[:],
                        o2t[p][:, sub * 128:(sub + 1) * 128],
                        wo_sb[:, p * D + oc * 512:p * D + (oc + 1) * 512],
                        start=(p == 0), stop=(p == 1))
                    if p == 0 and anchor is not None:
                        add_dep_helper(mm.ins, anchor.ins, sync=False,
                                       reason="interleave outproj after S")
                nc.vector.tensor_copy(
                    ost[:, oc * 512:(oc + 1) * 512], pp[:])
                if oc == 1:
                    nc.sync.dma_start(
                        pout[q0 + sub * 128:q0 + (sub + 1) * 128, :],
                        ost[:])
                    del ost_live[(qc, sub)]

            def emit_outproj(qc, o2t, anchor=None):
                for sub in range(4):
                    for oc in range(2):
                        emit_outproj_group(qc, o2t, sub, oc, anchor)

            po_live = {}

            def attn_s_exp(qc, p, kb):
                # S^T matmul pair then exp for one key block
                k0 = kb * KB
                ps = psS.tile([128, 2 * QC], F32, tag="s", name=f"s_{qc}_{p}_{kb}")
                s_anchor = nc.tensor.matmul(
                    ps[:, 0:QC],
                    kt[p][0:64, k0:k0 + KB],
                    qt[p][qc][0:64, :],
                    start=True, stop=True, tile_position=(0, 0))
                nc.tensor.matmul(
                    ps[:, QC:2 * QC],
                    kt[p][64:128, k0:k0 + KB],
                    qt[p][qc][64:128, :],
                    start=True, stop=True, tile_position=(64, 0))
                at = atp.tile([128, 2 * QC], BF16, tag="at", name=f"at_{qc}_{p}_{kb}")
                nc.scalar.activation(
                    at[:], ps[:], mybir.ActivationFunctionType.Exp)
                return at, s_anchor

            def attn_v(qc, p, kb, at):
                po = po_live[(qc, p)]
                for h in range(2):
                    nc.tensor.matmul(
                        po[h][:],
                        v2[p][kb][:, h * 65:h * 65 + 65],
                        at[:, h * QC:(h + 1) * QC],
                        start=(kb == 0), stop=(kb == NKB - 1))

            def attn_kb(qc, p, kb, pending):
                at, s_anchor = attn_s_exp(qc, p, kb)
                attn_v(qc, p, kb, at)
                # interleave 7 of the previous qc's 8 outproj groups from kb=3
                # on (anchor later than kb=1 so the normalization chain of the
                # previous pair has slack); the 8th is emitted after the loop
                if p == 1 and pending is not None and kb >= 3 and kb % 2 == 1:
                    idx = (kb - 3) // 2
                    pqc, po2t = pending
                    emit_outproj_group(pqc, po2t, idx // 2, idx % 2, s_anchor)

            def attn_norm(qc, p, o2t):
                # softmax normalization for both heads of pair p
                # (po rows 0:64 = O^T, row 64 = denominator)
                po = po_live.pop((qc, p))
                with tc.high_priority():
                    ots, rbs = [], []
                    for h in range(2):
                        # denominator chain first -- it has the longest
                        # latency (recip -> gpsimd broadcast) before the
                        # final multiply
                        den = smallp.tile([1, QC], F32, tag=f"den{h}", name=f"den_{qc}_{p}_{h}")
                        nc.vector.tensor_copy(den[:], po[h][64:65, :])
                        r = smallp.tile([1, QC], F32, tag=f"r{h}", name=f"r_{qc}_{p}_{h}")
                        nc.vector.reciprocal_approx_fast(r[:], den[:])
                        rb = smallp.tile([64, QC], F32, tag=f"rb{h}", name=f"rb_{qc}_{p}_{h}")
                        nc.gpsimd.partition_broadcast(rb[:], r[:])
                        rbs.append(rb)
                    for h in range(2):
                        ot = smallp.tile([64, QC], F32, tag=f"ot{h}", name=f"ot_{qc}_{p}_{h}")
                        nc.vector.tensor_copy(ot[:], po[h][0:64, :])
                        ots.append(ot)
                    for h in range(2):
                        nc.vector.tensor_mul(
                            o2t[h * 64:(h + 1) * 64, :],
                            ots[h][:], rbs[h][:])

            def new_po(qc, p):
                po_live[(qc, p)] = [
                    psO.tile([65, QC], F32, tag=f"o{h}", name=f"po_{qc}_{p}_{h}")
                    for h in range(2)]

            # ---- emission schedule ----
            # prologue: first K and Q slices, then qc0-p0 attention groups
            # interleaved with remaining K slices and V blocks; Q s1-3 are
            # emitted later as PE filler during steady-state attention
            proj_fm_slice(xk_t, wk_sb, 0, evac_kt)
            proj_fm_slice(xq_t, wq_sb, 0, evac_qt)

            o2t_cur = [o2tp.tile([128, QC], F16, tag=f"o2t{p}", name=f"o2t_0_{p}")
                       for p in range(2)]
            new_po(0, 0)
            for g in range(4):
                # S+exp first (no xv dependency -- keeps ACT streaming even
                # while xv data is still arriving), then V blocks, then attnV
                ats = [attn_s_exp(0, 0, kb)[0] for kb in range(4 * g, 4 * g + 4)]
                for b in range(4 * g, 4 * g + 4):
                    proj_v_block(b)
                for kb, at in zip(range(4 * g, 4 * g + 4), ats):
                    attn_v(0, 0, kb, at)
                if g < 3:
                    proj_fm_slice(xk_t, wk_sb, g + 1, evac_kt)
                else:
                    proj_fm_slice(xq_t, wq_sb, 1, evac_qt)
            attn_norm(0, 0, o2t_cur[0])
            new_po(0, 1)
            for kb in range(NKB):
                attn_kb(0, 1, kb, None)
            attn_norm(0, 1, o2t_cur[1])
            pending = (0, o2t_cur)

            # qc=1..3 with previous qc's outproj interleaved into pair 1
            # (7 groups anchored in the kb loop, the 8th right after)
            for qc in range(1, NQ):
                o2t_cur = [o2tp.tile([128, QC], F16, tag=f"o2t{p}", name=f"o2t_{qc}_{p}")
                           for p in range(2)]
                for p in range(2):
                    new_po(qc, p)
                    for kb in range(NKB):
                        attn_kb(qc, p, kb, pending if p == 1 else None)
                    if p == 1:
                        emit_outproj_group(pending[0], pending[1], 3, 1, None)
                    attn_norm(qc, p, o2t_cur[p])
                    if qc == 1:
                        proj_fm_slice(xq_t, wq_sb, 2 + p, evac_qt)
                pending = (qc, o2t_cur)
            emit_outproj(*pending)

    nc.compile()
    nc.m = get_hw_module(nc.m)
    return nc


def _pack_w(w_pair):
    # w_pair: [2, 1024, 64] -> [1024, 128] -> chunk-major [128, 8*128]
    w = np.concatenate([w_pair[0], w_pair[1]], axis=1)          # [1024, 128]
    return np.ascontiguousarray(
        w.reshape(ND, 128, 128).transpose(1, 0, 2).reshape(128, D))


def _pack_wv(w4):
    # w4: [4, 1024, 64] -> [1024, 256] -> chunk-major [128, 8*256]
    w = np.concatenate([w4[h] for h in range(4)], axis=1)       # [1024, 256]
    return np.ascontiguousarray(
        w.reshape(ND, 128, 256).transpose(1, 0, 2).reshape(128, ND * 256))


def _pack_wo(wo_pair):
    # wo_pair: [2, 64, 1024] -> [128, 1024]
    return np.ascontiguousarray(np.concatenate([wo_pair[0], wo_pair[1]], axis=0))


def kernel(q, k, v, W_query, W_key, W_val, W_out, _trace=False):
    q = np.asarray(q, dtype=np.float32)
    k = np.asarray(k, dtype=np.float32)
    v = np.asarray(v, dtype=np.float32)
    W_query = np.asarray(W_query, dtype=np.float32)
    W_key = np.asarray(W_key, dtype=np.float32)
    W_val = np.asarray(W_val, dtype=np.float32)
    W_out = np.asarray(W_out, dtype=np.float32)

    if "nc" not in _CACHE:
        _CACHE["nc"] = _build()
    nc = _CACHE["nc"]

    norm = 1.0 / np.sqrt(E)
    xT = {}
    for b in range(2):
        xT[("q", b)] = np.ascontiguousarray(q[b].T).astype(np.float16)
        xT[("k", b)] = np.ascontiguousarray(k[b].T).astype(np.float16)
        xT[("v", b)] = np.ascontiguousarray(v[b].T).astype(np.float16)

    in_maps = []
    for c in range(N_CORES):
        b, g = c // 4, c % 4
        hs = [4 * g, 4 * g + 1, 4 * g + 2, 4 * g + 3]
        m = {
            "xqT": xT[("q", b)], "xkT": xT[("k", b)], "xvT": xT[("v", b)],
            "wvm": _pack_wv(W_val[hs]).astype(np.float16),
            "wq": np.concatenate(
                [_pack_w(W_query[hs[2 * p:2 * p + 2]] * norm) for p in range(2)],
                axis=1).astype(np.float16),
            "wk": np.concatenate(
                [_pack_w(W_key[hs[2 * p:2 * p + 2]]) for p in range(2)],
                axis=1).astype(np.float16),
            "wo": np.concatenate(
                [_pack_wo(W_out[hs[2 * p:2 * p + 2]]) for p in range(2)],
                axis=1).astype(np.float16),
        }
        in_maps.append(m)

    res = run_bass_kernel_spmd(nc, in_maps, list(range(N_CORES)),
                               trace=_trace)
    parts = [res.results[c]["pout"].astype(np.float32) for c in range(N_CORES)]
    out = np.stack([
        parts[0] + parts[1] + parts[2] + parts[3],
        parts[4] + parts[5] + parts[6] + parts[7],
    ]).astype(np.float32)
    if _trace:
        _CACHE["last_result"] = res
    return out
